# revision 43
# baseline (speedup 1.0000x reference)
"""Trainium2 Bass kernel for nn_Attention_14370960572643 (gnn_message_passing).

Math (per batch b):
  local_pair[b,i,j,:] = local[b,i,:] + local[b,j,:]
  att  = relu(concat(local_pair, binary) @ W1 + b1)        [B,N,N,H]
  score = sigmoid(att @ W2 + b2)                            [B,N,N,1]
  G[b,i,:] = sum_j local[b,j,:] * score[b,i,j]              [B,N,H]
  outputs (E sparse pairs): lp[e] = local[bb,ii]+local[bb,jj]
                            gp[e] = G[bb,ii]+G[bb,jj]

v3 structure:
  * Device computes ONLY att -> score -> G.  The sparse outputs lp/gp are
    pure index-gathers: lp needs only the input local_feats, gp needs only
    the tiny G [B,N,H]; both are assembled host-side after the run.  This
    removes the one-hot gather matmuls, their PSUM->SBUF copies, the oh
    DMA-in and the big lp/gp DMA-outs entirely.
  * att collapses to one K=112 fp8 DoubleRow matmul per (chunk, h-tile):
    contraction rows = 100 P rows (P = local @ W1[:H]) + 1 ones row (b1)
    + 11 W1b rows.  The moving operand packs BOTH pair indicators into the
    identity rows (rhs[r, col] = [r==j(col)] + [r==i(col)]).  The indicator
    part (K 0..100) is batch-independent: ONE SBUF tile holds it, loaded
    once; only the 11 binary rows (slab1 partitions 37..47) are re-DMAed
    between batches.
  * PSUM layout amortizes drain overhead: per 500-col chunk kt0/kt1 land in
    one [128,2,512] f32 tile (one [128,2,500] relu-drain, free=1000); the
    44-row kt2 tiles of 4 consecutive chunks pack into one [128,2,512] bank
    pair at partition offsets 0/44 (one [88,2,500] drain per quad).  50
    drains / 50k free-elements per core, greedily balanced ACT vs DVE.
  * score matmuls are out-free=1 (out = psc[0:100, i:i+1], lhsT = attc
    column block, rhs = W2 tile): ~0 engine cost.  sigmoid -> scT -> G.
"""

import numpy as np

B, N, H, BIN = 16, 100, 300, 11
NN2 = N * N                  # 10000 pair columns per batch
NCORES = 8
BPC = B // NCORES            # batches per core
CH_I = 5                     # i values per chunk
CH = CH_I * N                # 500 pair columns per chunk
NCH = N // CH_I              # 20 chunks per batch
H_T = [(0, 128), (128, 128), (256, 44)]     # h tiles
K112 = N + 1 + BIN           # 112 contraction rows
K64 = 64                     # DoubleRow slab partitions (112 padded to 128)
WSCALE = 16.0                # W1b x16 in C, binary /16 in rhs (fp8 range)

_CACHE = {}


def _build_nc():
    import concourse.bass as bass
    import concourse.mybir as mybir
    import concourse.tile as tile
    from concourse import bacc

    dt = mybir.dt
    f32 = dt.float32
    bf16 = dt.bfloat16
    fp8t = dt.float8e4

    nc = bacc.Bacc("TRN2", target_bir_lowering=False, debug=False,
                   num_devices=NCORES)

    # ---- dram parameters (per-core shards) ----
    # mega-const: W1a (3x300) | localT (6x100) | lnat (2x300) | W2c (3x1)
    mc_d = nc.dram_tensor("MC", [128, 2103], bf16, kind="ExternalInput").ap()
    ind_d = nc.dram_tensor("IND", [K64, 2, NN2], fp8t,
                           kind="ExternalInput").ap()
    bin_d = nc.dram_tensor("BINF", [BPC, BIN, NN2], fp8t,
                           kind="ExternalInput").ap()
    cconst_d = nc.dram_tensor("Cconst", [1 + BIN, 3, 128], fp8t,
                              kind="ExternalInput").ap()
    b2_d = nc.dram_tensor("b2", [1, 1], f32, kind="ExternalInput").ap()
    g_d = nc.dram_tensor("G", [BPC * N, H], bf16, kind="ExternalOutput").ap()

    Relu = mybir.ActivationFunctionType.Relu
    Sigmoid = mybir.ActivationFunctionType.Sigmoid
    DR = mybir.MatmulPerfMode.DoubleRow

    with tile.TileContext(nc) as tc:
        with (
            tc.tile_pool(name="const", bufs=1) as cpool,
            tc.tile_pool(name="attca", bufs=12) as attap,
            tc.tile_pool(name="attcs", bufs=6) as attsp,
            tc.tile_pool(name="paA", bufs=2, space="PSUM") as paA_pool,
            tc.tile_pool(name="paS", bufs=1, space="PSUM") as paS_pool,
            tc.tile_pool(name="ppg", bufs=1, space="PSUM") as pg_pool,
            tc.tile_pool(name="psc", bufs=1, space="PSUM") as psc_pool,
        ):
            # ---------- SBUF constants ----------
            mc = cpool.tile([128, 2103], bf16, tag="mc", name="mc")
            nc.sync.dma_start(out=mc[:, 0:1500], in_=mc_d[:, 0:1500])
            W1a_sb = [mc[0:kk, kt * H:(kt + 1) * H]
                      for kt, (k0, kk) in enumerate(H_T)]
            localT_sb = [[mc[0:kk, 900 + (b * 3 + kt) * N:
                             900 + (b * 3 + kt + 1) * N]
                          for kt, (k0, kk) in enumerate(H_T)]
                         for b in range(BPC)]
            lnat_sb = [mc[0:N, 1500 + b * H:1500 + (b + 1) * H]
                       for b in range(BPC)]
            W2c_sb = [mc[0:hh, 2100 + kt:2101 + kt]
                      for kt, (h0, hh) in enumerate(H_T)]
            b2rep = cpool.tile([128, 1], f32, tag="b2rep", name="b2rep")
            # dummy sigmoid+relu at warmup (fed by memset, no DMA dep) pin
            # the act tables before the drain stream starts
            _junk = cpool.tile([1, 2], f32, tag="junk", name="junk")
            nc.vector.memset(_junk[:], 0.0)
            nc.scalar.activation(_junk[:, 0:1], _junk[:, 1:2], Sigmoid)
            nc.scalar.activation(_junk[:, 0:1], _junk[:, 1:2], Relu)
            # PE p-state warmer: touch the PE early so the 2.4GHz ramp
            # (3us of busy history) completes before the chunk stream
            _wsb = cpool.tile([1, 8], bf16, tag="wsb", name="wsb")
            nc.vector.memset(_wsb[:], 0.0)
            _wps = pg_pool.tile([128, 512], f32, tag="pg", name="wps")
            for _i in range(12):
                nc.tensor.matmul(out=_wps[0:1, 0:8], lhsT=_wsb[:, 0:1],
                                 rhs=_wsb[:], start=True, stop=True)

            # per-batch rhs (batch streams interleave, so each batch needs
            # its own binary rows; the indicator part is DMAed twice)
            rhs_sb = [cpool.tile([K64, 2, NN2], fp8t, tag=f"rhs{b}",
                                 name=f"rhs{b}") for b in range(BPC)]
            # per-batch stationary C (fp8 DoubleRow layout, one tile per
            # batch: [K64, slab, kt, 128], kt2 zero-padded past col 44)
            C_sb = []
            scT_sb, g16_sb = [], []
            for b in range(BPC):
                C_sb.append(cpool.tile([K64, 2, 3, 128], fp8t,
                                       tag=f"c{b}", name=f"c{b}"))
                scT_sb.append(cpool.tile([N, N], bf16, tag=f"sct{b}",
                                         name=f"sct{b}"))
                g16_sb.append(cpool.tile([N, H], bf16, tag=f"g16_{b}",
                                         name=f"g16_{b}"))

            def load_ind(b, lo, hi):
                # skip the binary rows (slab1 partitions 37..47) so BIN
                # loads don't order against IND loads.
                sl = slice(lo, hi)
                nc.sync.dma_start(out=rhs_sb[b][:, 0, sl],
                                  in_=ind_d[:, 0, sl])
                nc.sync.dma_start(out=rhs_sb[b][0:37, 1, sl],
                                  in_=ind_d[0:37, 1, sl])

            def load_zero(b):
                # pad rows (uninitialized fp8 can hold NaN; 0 x NaN = NaN):
                # one whole-width DMA per batch, early
                nc.sync.dma_start(out=rhs_sb[b][48:64, 1, :],
                                  in_=ind_d[48:64, 1, :])

            def load_bin(b):
                # Pool-issued (SWDGE): keeps HWDGE free for the IND pieces
                nc.gpsimd.dma_start(out=rhs_sb[b][37:48, 1, :],
                                    in_=bin_d[b][:, :])

            def load_cconst(b):
                # whole-tile zero (covers kt2 col pad + slab pads), then
                # rows 100..111 = slab1 partitions 36..47 from dram.
                # cconst rides SP/HWDGE so the Pool queue stays short.
                nc.gpsimd.memset(C_sb[b][:, :, :, :], 0.0)
                nc.sync.dma_start(out=C_sb[b][36:48, 1, :, :],
                                  in_=cconst_d[:, :, :])

            def p_stage(b):
                # P-stages ride startup-idle paA slots so the two batches'
                # stages run in parallel instead of chaining on one bank
                psm3 = paA_pool.tile([128, 2, 512], f32, tag="a",
                                     name=f"psp{b}")
                ps = psm3[0:N, 0, 0:H]
                for kt in range(3):
                    nc.tensor.matmul(out=ps[:], lhsT=localT_sb[b][kt][:],
                                     rhs=W1a_sb[kt][:],
                                     start=(kt == 0), stop=(kt == 2))
                ps2 = psm3[0:N, 0, 0:256].rearrange("p (t c) -> p t c", t=2)
                nc.vector.tensor_copy(out=C_sb[b][0:64, 0, 0:2, 0:128],
                                      in_=ps2[0:64, :, :])
                nc.vector.tensor_copy(out=C_sb[b][0:64, 0, 2, 0:44],
                                      in_=psm3[0:64, 0, 256:300])
                nc.scalar.copy(out=C_sb[b][0:36, 1, 0:2, 0:128],
                               in_=ps2[64:100, :, :])
                nc.scalar.copy(out=C_sb[b][0:36, 1, 2, 0:44],
                               in_=psm3[64:100, 0, 256:300])

            # ---- engine-balanced drain assignment ----
            # greedy: assign each drain to the engine with less accumulated
            # time.  ACT: 0.8333 ns/elem + 185; DVE: 1.0417 ns/elem + 125.
            acc = {"act": 2200.0, "dve": 1500.0}  # bias: ACT sigmoids etc.

            def drain(out_ap, in_ap, nfree):
                t_act = nfree * 0.8333 + 185.0
                t_dve = nfree * 1.0417 + 125.0
                if acc["act"] + t_act <= acc["dve"] + t_dve:
                    acc["act"] += t_act
                    nc.scalar.activation(out_ap, in_ap, Relu)
                else:
                    acc["dve"] += t_dve
                    nc.vector.tensor_scalar_max(out=out_ap, in0=in_ap,
                                                scalar1=0.0)

            def emit_score_group(b, psc, attca, attcs, c, s):
                i = c * CH_I + s
                nc.tensor.matmul(
                    out=psc[0:N, b, i:i + 1],
                    lhsT=attca[0:128, 0, s * N:(s + 1) * N],
                    rhs=W2c_sb[0][:], start=True, stop=False)
                nc.tensor.matmul(
                    out=psc[0:N, b, i:i + 1],
                    lhsT=attca[0:128, 1, s * N:(s + 1) * N],
                    rhs=W2c_sb[1][:], start=False, stop=False)
                nc.tensor.matmul(
                    out=psc[0:N, b, i:i + 1],
                    lhsT=attcs[0:44, b, s * N:(s + 1) * N],
                    rhs=W2c_sb[2][:], start=False, stop=True)

            SIG_CUTS = [(0, 64), (64, 96), (96, N)]

            def emit_sig_g(b, psc, piece):
                """sigmoid + G matmul + g16 copy + out-DMA for an i-range;
                pieces fire as scores accumulate so only the tiny last
                piece sits on the tail."""
                i0, i1 = SIG_CUTS[piece]
                nc.scalar.activation(scT_sb[b][:, i0:i1],
                                     psc[0:N, b, i0:i1], Sigmoid,
                                     bias=b2rep[0:N, :])
                psm = pg_pool.tile([128, 512], f32, tag="pg",
                                   name=f"psg{b}_{piece}")
                nc.tensor.matmul(out=psm[0:i1 - i0, 0:H],
                                 lhsT=scT_sb[b][:, i0:i1],
                                 rhs=lnat_sb[b][:], start=True, stop=True)
                nc.vector.tensor_copy(out=g16_sb[b][i0:i1, :],
                                      in_=psm[0:i1 - i0, 0:H])
                nc.gpsimd.dma_start(
                    out=g_d[b * N + i0:b * N + i1, :],
                    in_=g16_sb[b][i0:i1, :])

            # ------------- interleaved two-stream schedule -------------
            # startup loads.  HWDGE order: MC piece 0 (P-stage), zero-pad
            # rows, first IND pieces of both batches, cconst, the rest.
            # Small loads ride Pool SWDGE.
            nc.sync.dma_start(out=b2rep[:],
                              in_=b2_d[0:1, :].to_broadcast([128, 1]))
            load_bin(0)
            load_bin(1)
            load_zero(0)
            load_zero(1)
            load_ind(0, 0, 1000)
            load_ind(1, 0, 1000)
            load_cconst(0)
            load_cconst(1)
            # lnat / W2c piece of the mega-const (needed once scores start)
            nc.sync.dma_start(out=mc[:, 1500:2103], in_=mc_d[:, 1500:2103])
            p_stage(0)
            p_stage(1)
            for (lo, hi) in [(1000, 4000), (4000, 7000), (7000, 10000)]:
                load_ind(0, lo, hi)
                load_ind(1, lo, hi)

            psc_t = psc_pool.tile([128, 2, 128], f32, tag="sc", name="sc")
            psc = [psc_t for b in range(BPC)]
            attca_t = {}
            squeue = []          # (b, c, s) score groups not yet emitted
            emitted = 0
            sig_done = [0, 0]

            attcs_t = {}

            def emit_scores(upto):
                nonlocal emitted
                while squeue and emitted < upto:
                    bb_, cc, s = squeue.pop(0)
                    emit_score_group(bb_, psc[bb_], attca_t[(bb_, cc)],
                                     attcs_t[cc], cc, s)
                    emitted += 1
                    # fire sigmoid pieces as soon as each accumulates
                    i_done = cc * CH_I + s
                    while (sig_done[bb_] < len(SIG_CUTS) and
                           i_done == SIG_CUTS[sig_done[bb_]][1] - 1):
                        emit_sig_g(bb_, psc[bb_], sig_done[bb_])
                        sig_done[bb_] += 1

            for c in range(NCH):
                c0 = c * CH
                paS = paS_pool.tile([128, 2, 512], f32, tag="s",
                                    name=f"paS{c}")
                for b in range(BPC):
                    paA = paA_pool.tile([128, 2, 512], f32, tag="a",
                                        name=f"paA{b}_{c}")
                    for kt in range(2):
                        nc.tensor.matmul(
                            out=paA[0:128, kt, 0:CH],
                            lhsT=C_sb[b][:, :, kt, 0:128],
                            rhs=rhs_sb[b][:, :, c0:c0 + CH],
                            start=True, stop=True, perf_mode=DR)
                    nc.tensor.matmul(
                        out=paS[0:64, b, 0:CH],
                        lhsT=C_sb[b][:, :, 2, 0:64],
                        rhs=rhs_sb[b][:, :, c0:c0 + CH],
                        start=True, stop=True, perf_mode=DR)
                    attca = attap.tile([128, 2, CH], bf16, tag="attca",
                                       name=f"attca{b}_{c}")
                    attca_t[(b, c)] = attca
                    drain(attca[:, :, :], paA[:, :, 0:CH], 2 * CH)
                    for s in range(CH_I):
                        squeue.append((b, c, s))
                attcs = attsp.tile([64, 2, CH], bf16, tag="attcs",
                                   name=f"attcs{c}")
                attcs_t[c] = attcs
                drain(attcs[:, :, :], paS[0:64, :, 0:CH], 2 * CH)
                # keep scores ~2 chunk-slots behind the matmul stream
                emit_scores((c - 1) * BPC * CH_I)
            emit_scores(10**9)

    nc.compile()
    return nc


def _prep_inputs(local_feats, binary_feats, W1, b1, W2, b2):
    """Build per-core in_maps. Host-side layout only."""
    import ml_dtypes
    bf = ml_dtypes.bfloat16
    f8 = ml_dtypes.float8_e4m3
    local_feats = np.ascontiguousarray(local_feats, dtype=np.float32)
    binary_feats = np.ascontiguousarray(binary_feats, dtype=np.float32)
    W1 = np.ascontiguousarray(W1, dtype=np.float32)
    b1 = np.ascontiguousarray(b1, dtype=np.float32).reshape(1, H)
    W2 = np.ascontiguousarray(W2, dtype=np.float32).reshape(H, 1)
    b2 = np.ascontiguousarray(b2, dtype=np.float32).reshape(1, 1)

    # IND: rows 0..99 = [r==j]+[r==i]; row 100 = ones (b1 row); DR layout
    cols = np.arange(NN2)
    ind2 = np.zeros((N + 1, NN2), dtype=np.float32)
    np.add.at(ind2, (cols % N, cols), 1.0)
    np.add.at(ind2, (cols // N, cols), 1.0)
    ind2[N, :] = 1.0
    ind128 = np.concatenate(
        [ind2, np.zeros((128 - (N + 1), NN2), np.float32)], axis=0)
    ind_dr = np.ascontiguousarray(
        ind128.reshape(2, K64, NN2).transpose(1, 0, 2)).astype(f8)

    cc300 = np.concatenate([b1, W1[H:] * WSCALE], axis=0)    # [12, 300]
    cconst = np.zeros((1 + BIN, 3, 128), dtype=np.float32)
    for kt, (h0, hh) in enumerate(H_T):
        cconst[:, kt, 0:hh] = cc300[:, h0:h0 + hh]
    cconst = cconst.astype(f8)

    in_maps = []
    for c in range(NCORES):
        sl = slice(c * BPC, c * BPC + BPC)
        binT = np.ascontiguousarray(
            binary_feats[sl].transpose(0, 3, 1, 2).reshape(BPC, BIN, NN2)
            / WSCALE).astype(f8)
        mc = np.zeros((128, 2103), dtype=np.float32)
        localT = local_feats[sl].transpose(0, 2, 1)          # [BPC, H, N]
        for kt, (k0, kk) in enumerate(H_T):
            mc[0:kk, kt * H:(kt + 1) * H] = W1[k0:k0 + kk, :H]
            for b in range(BPC):
                mc[0:kk, 900 + (b * 3 + kt) * N:900 + (b * 3 + kt + 1) * N] = \
                    localT[b, k0:k0 + kk, :]
            mc[0:kk, 2100 + kt] = W2[k0:k0 + kk, 0]
        for b in range(BPC):
            mc[0:N, 1500 + b * H:1500 + (b + 1) * H] = \
                local_feats[sl][b].reshape(N, H)
        in_maps.append({
            "MC": mc.astype(bf),
            "IND": ind_dr,
            "BINF": binT,
            "Cconst": cconst,
            "b2": b2,
        })
    return in_maps


def _run(in_maps, trace=False):
    from concourse.bass_utils import run_bass_kernel_spmd
    if "nc" not in _CACHE:
        _CACHE["nc"] = _build_nc()
    nc = _CACHE["nc"]
    _CACHE["last_nc"] = nc
    res = run_bass_kernel_spmd(nc, in_maps, core_ids=list(range(NCORES)),
                               trace=trace)
    return res


def kernel(local_feats, binary_feats, sparse_idx, W1, b1, W2, b2):
    local_feats = np.ascontiguousarray(local_feats, dtype=np.float32)
    in_maps = _prep_inputs(local_feats, binary_feats, W1, b1, W2, b2)
    res = _run(in_maps)
    G = np.zeros((B, N, H), dtype=np.float32)
    for c in range(NCORES):
        G[c * BPC:(c + 1) * BPC] = np.asarray(
            res.results[c]["G"], dtype=np.float32).reshape(BPC, N, H)
    sparse_idx = np.asarray(sparse_idx)
    bb = sparse_idx[:, 0].astype(np.int64)
    ii = sparse_idx[:, 1].astype(np.int64)
    jj = sparse_idx[:, 2].astype(np.int64)
    lp = local_feats[bb, ii] + local_feats[bb, jj]
    gp = G[bb, ii] + G[bb, jj]
    return (lp, gp)


# revision 47
# speedup vs baseline: 1.0285x; 1.0285x over previous
"""Trainium2 Bass kernel for nn_Attention_14370960572643 (gnn_message_passing).

Math (per batch b):
  local_pair[b,i,j,:] = local[b,i,:] + local[b,j,:]
  att  = relu(concat(local_pair, binary) @ W1 + b1)        [B,N,N,H]
  score = sigmoid(att @ W2 + b2)                            [B,N,N,1]
  G[b,i,:] = sum_j local[b,j,:] * score[b,i,j]              [B,N,H]
  outputs (E sparse pairs): lp[e] = local[bb,ii]+local[bb,jj]
                            gp[e] = G[bb,ii]+G[bb,jj]

v3 structure:
  * Device computes ONLY att -> score -> G.  The sparse outputs lp/gp are
    pure index-gathers: lp needs only the input local_feats, gp needs only
    the tiny G [B,N,H]; both are assembled host-side after the run.  This
    removes the one-hot gather matmuls, their PSUM->SBUF copies, the oh
    DMA-in and the big lp/gp DMA-outs entirely.
  * att collapses to one K=112 fp8 DoubleRow matmul per (chunk, h-tile):
    contraction rows = 100 P rows (P = local @ W1[:H]) + 1 ones row (b1)
    + 11 W1b rows.  The moving operand packs BOTH pair indicators into the
    identity rows (rhs[r, col] = [r==j(col)] + [r==i(col)]).  The indicator
    part (K 0..100) is batch-independent: ONE SBUF tile holds it, loaded
    once; only the 11 binary rows (slab1 partitions 37..47) are re-DMAed
    between batches.
  * PSUM layout amortizes drain overhead: per 500-col chunk kt0/kt1 land in
    one [128,2,512] f32 tile (one [128,2,500] relu-drain, free=1000); the
    44-row kt2 tiles of 4 consecutive chunks pack into one [128,2,512] bank
    pair at partition offsets 0/44 (one [88,2,500] drain per quad).  50
    drains / 50k free-elements per core, greedily balanced ACT vs DVE.
  * score matmuls are out-free=1 (out = psc[0:100, i:i+1], lhsT = attc
    column block, rhs = W2 tile): ~0 engine cost.  sigmoid -> scT -> G.
"""

import numpy as np

B, N, H, BIN = 16, 100, 300, 11
NN2 = N * N                  # 10000 pair columns per batch
NCORES = 8
BPC = B // NCORES            # batches per core
CH_I = 5                     # i values per chunk
CH = CH_I * N                # 500 pair columns per chunk
NCH = N // CH_I              # 20 chunks per batch
H_T = [(0, 128), (128, 128), (256, 44)]     # h tiles
K112 = N + 1 + BIN           # 112 contraction rows
K64 = 64                     # DoubleRow slab partitions (112 padded to 128)
WSCALE = 16.0                # W1b x16 in C, binary /16 in rhs (fp8 range)

_CACHE = {}


def _build_nc():
    import concourse.bass as bass
    import concourse.mybir as mybir
    import concourse.tile as tile
    from concourse import bacc

    dt = mybir.dt
    f32 = dt.float32
    bf16 = dt.bfloat16
    fp8t = dt.float8e4

    nc = bacc.Bacc("TRN2", target_bir_lowering=False, debug=False,
                   num_devices=NCORES)

    # ---- dram parameters (per-core shards) ----
    # mega-const: W1a (3x300) | localT (6x100) | lnat (2x300) | W2c (3x1)
    mc_d = nc.dram_tensor("MC", [128, 2103], bf16, kind="ExternalInput").ap()
    ind_d = nc.dram_tensor("IND", [K64, 2, NN2], fp8t,
                           kind="ExternalInput").ap()
    # binary rows + zero-pad rows (slab1 partitions 37..63) in one block
    bin_d = nc.dram_tensor("BINF", [BPC, 27, NN2], fp8t,
                           kind="ExternalInput").ap()
    cconst_d = nc.dram_tensor("Cconst", [1 + BIN, 3, 128], fp8t,
                              kind="ExternalInput").ap()
    b2_d = nc.dram_tensor("b2", [1, 1], f32, kind="ExternalInput").ap()
    g_d = nc.dram_tensor("G", [BPC * N, H], bf16, kind="ExternalOutput").ap()

    Relu = mybir.ActivationFunctionType.Relu
    Sigmoid = mybir.ActivationFunctionType.Sigmoid
    DR = mybir.MatmulPerfMode.DoubleRow

    with tile.TileContext(nc) as tc:
        with (
            tc.tile_pool(name="const", bufs=1) as cpool,
            tc.tile_pool(name="attca", bufs=12) as attap,
            tc.tile_pool(name="attcs", bufs=6) as attsp,
            tc.tile_pool(name="paA", bufs=2, space="PSUM") as paA_pool,
            tc.tile_pool(name="paS", bufs=1, space="PSUM") as paS_pool,
            tc.tile_pool(name="ppg", bufs=1, space="PSUM") as pg_pool,
            tc.tile_pool(name="psc", bufs=1, space="PSUM") as psc_pool,
        ):
            # ---------- SBUF constants ----------
            mc = cpool.tile([128, 2103], bf16, tag="mc", name="mc")
            nc.sync.dma_start(out=mc[:, 0:1500], in_=mc_d[:, 0:1500])
            W1a_sb = [mc[0:kk, kt * H:(kt + 1) * H]
                      for kt, (k0, kk) in enumerate(H_T)]
            localT_sb = [[mc[0:kk, 900 + (b * 3 + kt) * N:
                             900 + (b * 3 + kt + 1) * N]
                          for kt, (k0, kk) in enumerate(H_T)]
                         for b in range(BPC)]
            lnat_sb = [mc[0:N, 1500 + b * H:1500 + (b + 1) * H]
                       for b in range(BPC)]
            W2c_sb = [mc[0:hh, 2100 + kt:2101 + kt]
                      for kt, (h0, hh) in enumerate(H_T)]
            b2rep = cpool.tile([128, 1], f32, tag="b2rep", name="b2rep")
            # dummy sigmoid+relu at warmup (fed by memset, no DMA dep) pin
            # the act tables before the drain stream starts
            _junk = cpool.tile([1, 2], f32, tag="junk", name="junk")
            nc.vector.memset(_junk[:], 0.0)
            nc.scalar.activation(_junk[:, 0:1], _junk[:, 1:2], Sigmoid)
            nc.scalar.activation(_junk[:, 0:1], _junk[:, 1:2], Relu)
            # PE p-state warmer: touch the PE early so the 2.4GHz ramp
            # (3us of busy history) completes before the chunk stream
            _wsb = cpool.tile([1, 8], bf16, tag="wsb", name="wsb")
            nc.vector.memset(_wsb[:], 0.0)
            _wps = pg_pool.tile([128, 512], f32, tag="pg", name="wps")
            for _i in range(12):
                nc.tensor.matmul(out=_wps[0:1, 0:8], lhsT=_wsb[:, 0:1],
                                 rhs=_wsb[:], start=True, stop=True)

            # per-batch rhs (batch streams interleave, so each batch needs
            # its own binary rows; the indicator part is DMAed twice)
            rhs_sb = [cpool.tile([K64, 2, NN2], fp8t, tag=f"rhs{b}",
                                 name=f"rhs{b}") for b in range(BPC)]
            # per-batch stationary C (fp8 DoubleRow layout, one tile per
            # batch: [K64, slab, kt, 128], kt2 zero-padded past col 44)
            C_sb = []
            scT_sb, g16_sb = [], []
            for b in range(BPC):
                C_sb.append(cpool.tile([K64, 2, 3, 128], fp8t,
                                       tag=f"c{b}", name=f"c{b}"))
                scT_sb.append(cpool.tile([N, N], bf16, tag=f"sct{b}",
                                         name=f"sct{b}"))
                g16_sb.append(cpool.tile([N, H], bf16, tag=f"g16_{b}",
                                         name=f"g16_{b}"))

            def load_ind(b, lo, hi):
                # skip the binary rows (slab1 partitions 37..47) so BIN
                # loads don't order against IND loads.
                sl = slice(lo, hi)
                nc.sync.dma_start(out=rhs_sb[b][:, 0, sl],
                                  in_=ind_d[:, 0, sl])
                nc.sync.dma_start(out=rhs_sb[b][0:37, 1, sl],
                                  in_=ind_d[0:37, 1, sl])

            def load_bin(b):
                # binary + zero-pad rows in one Pool-issued (SWDGE) DMA;
                # the pad rows must be written (uninitialized fp8 can hold
                # NaN, and 0 x NaN = NaN in the PE)
                nc.gpsimd.dma_start(out=rhs_sb[b][37:64, 1, :],
                                    in_=bin_d[b][:, :])

            def load_cconst(b):
                # whole-tile zero (covers kt2 col pad + slab pads), then
                # rows 100..111 = slab1 partitions 36..47 from dram.
                # cconst rides SP/HWDGE so the Pool queue stays short.
                nc.gpsimd.memset(C_sb[b][:, :, :, :], 0.0)
                nc.sync.dma_start(out=C_sb[b][36:48, 1, :, :],
                                  in_=cconst_d[:, :, :])

            def p_stage(b):
                # P-stages ride startup-idle paA slots so the two batches'
                # stages run in parallel instead of chaining on one bank
                psm3 = paA_pool.tile([128, 2, 512], f32, tag="a",
                                     name=f"psp{b}")
                ps = psm3[0:N, 0, 0:H]
                for kt in range(3):
                    nc.tensor.matmul(out=ps[:], lhsT=localT_sb[b][kt][:],
                                     rhs=W1a_sb[kt][:],
                                     start=(kt == 0), stop=(kt == 2))
                ps2 = psm3[0:N, 0, 0:256].rearrange("p (t c) -> p t c", t=2)
                nc.vector.tensor_copy(out=C_sb[b][0:64, 0, 0:2, 0:128],
                                      in_=ps2[0:64, :, :])
                nc.vector.tensor_copy(out=C_sb[b][0:64, 0, 2, 0:44],
                                      in_=psm3[0:64, 0, 256:300])
                nc.scalar.copy(out=C_sb[b][0:36, 1, 0:2, 0:128],
                               in_=ps2[64:100, :, :])
                nc.scalar.copy(out=C_sb[b][0:36, 1, 2, 0:44],
                               in_=psm3[64:100, 0, 256:300])

            # ---- engine-balanced drain assignment ----
            # greedy: assign each drain to the engine with less accumulated
            # time.  ACT: 0.8333 ns/elem + 185; DVE: 1.0417 ns/elem + 125.
            acc = {"act": 2200.0, "dve": 1500.0}  # bias: ACT sigmoids etc.

            def drain(out_ap, in_ap, nfree):
                t_act = nfree * 0.8333 + 185.0
                t_dve = nfree * 1.0417 + 125.0
                if acc["act"] + t_act <= acc["dve"] + t_dve:
                    acc["act"] += t_act
                    nc.scalar.activation(out_ap, in_ap, Relu)
                else:
                    acc["dve"] += t_dve
                    nc.vector.tensor_scalar_max(out=out_ap, in0=in_ap,
                                                scalar1=0.0)

            def emit_score_group(b, psc, attca, attcs, c, s):
                i = c * CH_I + s
                nc.tensor.matmul(
                    out=psc[0:N, b, i:i + 1],
                    lhsT=attca[0:128, 0, s * N:(s + 1) * N],
                    rhs=W2c_sb[0][:], start=True, stop=False)
                nc.tensor.matmul(
                    out=psc[0:N, b, i:i + 1],
                    lhsT=attca[0:128, 1, s * N:(s + 1) * N],
                    rhs=W2c_sb[1][:], start=False, stop=False)
                nc.tensor.matmul(
                    out=psc[0:N, b, i:i + 1],
                    lhsT=attcs[0:44, b, s * N:(s + 1) * N],
                    rhs=W2c_sb[2][:], start=False, stop=True)

            SIG_CUTS = [(0, 64), (64, N)]

            def emit_sig_g(b, psc, piece):
                """sigmoid + G matmul + g16 copy + out-DMA for an i-range;
                pieces fire as scores accumulate so only the tiny last
                piece sits on the tail."""
                i0, i1 = SIG_CUTS[piece]
                nc.scalar.activation(scT_sb[b][:, i0:i1],
                                     psc[0:N, b, i0:i1], Sigmoid,
                                     bias=b2rep[0:N, :])
                psm = pg_pool.tile([128, 512], f32, tag="pg",
                                   name=f"psg{b}_{piece}")
                nc.tensor.matmul(out=psm[0:i1 - i0, 0:H],
                                 lhsT=scT_sb[b][:, i0:i1],
                                 rhs=lnat_sb[b][:], start=True, stop=True)
                nc.vector.tensor_copy(out=g16_sb[b][i0:i1, :],
                                      in_=psm[0:i1 - i0, 0:H])
                nc.gpsimd.dma_start(
                    out=g_d[b * N + i0:b * N + i1, :],
                    in_=g16_sb[b][i0:i1, :])

            # ------------- interleaved two-stream schedule -------------
            # startup loads.  HWDGE order: MC piece 0 (P-stage), zero-pad
            # rows, first IND pieces of both batches, cconst, the rest.
            # Small loads ride Pool SWDGE.
            load_bin(0)
            load_bin(1)
            load_cconst(0)
            load_cconst(1)
            nc.sync.dma_start(out=b2rep[:],
                              in_=b2_d[0:1, :].to_broadcast([128, 1]))
            load_ind(0, 0, 1000)
            load_ind(1, 0, 1000)
            # lnat / W2c piece of the mega-const (needed once scores start)
            nc.sync.dma_start(out=mc[:, 1500:2103], in_=mc_d[:, 1500:2103])
            p_stage(0)
            p_stage(1)
            for (lo, hi) in [(1000, 4000), (4000, 7000), (7000, 10000)]:
                load_ind(0, lo, hi)
                load_ind(1, lo, hi)

            psc_t = psc_pool.tile([128, 2, 128], f32, tag="sc", name="sc")
            psc = [psc_t for b in range(BPC)]
            attca_t = {}
            squeue = []          # (b, c, s) score groups not yet emitted
            emitted = 0
            sig_done = [0, 0]

            attcs_t = {}

            def emit_scores(upto):
                nonlocal emitted
                while squeue and emitted < upto:
                    bb_, cc, s = squeue.pop(0)
                    emit_score_group(bb_, psc[bb_], attca_t[(bb_, cc)],
                                     attcs_t[cc], cc, s)
                    emitted += 1
                    # fire sigmoid pieces as soon as each accumulates
                    i_done = cc * CH_I + s
                    while (sig_done[bb_] < len(SIG_CUTS) and
                           i_done == SIG_CUTS[sig_done[bb_]][1] - 1):
                        emit_sig_g(bb_, psc[bb_], sig_done[bb_])
                        sig_done[bb_] += 1

            for c in range(NCH):
                c0 = c * CH
                paS = paS_pool.tile([128, 2, 512], f32, tag="s",
                                    name=f"paS{c}")
                for b in range(BPC):
                    paA = paA_pool.tile([128, 2, 512], f32, tag="a",
                                        name=f"paA{b}_{c}")
                    for kt in range(2):
                        nc.tensor.matmul(
                            out=paA[0:128, kt, 0:CH],
                            lhsT=C_sb[b][:, :, kt, 0:128],
                            rhs=rhs_sb[b][:, :, c0:c0 + CH],
                            start=True, stop=True, perf_mode=DR)
                    nc.tensor.matmul(
                        out=paS[0:64, b, 0:CH],
                        lhsT=C_sb[b][:, :, 2, 0:64],
                        rhs=rhs_sb[b][:, :, c0:c0 + CH],
                        start=True, stop=True, perf_mode=DR)
                    attca = attap.tile([128, 2, CH], bf16, tag="attca",
                                       name=f"attca{b}_{c}")
                    attca_t[(b, c)] = attca
                    drain(attca[:, :, :], paA[:, :, 0:CH], 2 * CH)
                    for s in range(CH_I):
                        squeue.append((b, c, s))
                attcs = attsp.tile([64, 2, CH], bf16, tag="attcs",
                                   name=f"attcs{c}")
                attcs_t[c] = attcs
                drain(attcs[:, :, :], paS[0:64, :, 0:CH], 2 * CH)
                # keep scores ~2 chunk-slots behind the matmul stream
                emit_scores((c - 1) * BPC * CH_I)
            emit_scores(10**9)

    nc.compile()
    return nc


def _prep_inputs(local_feats, binary_feats, W1, b1, W2, b2):
    """Build per-core in_maps. Host-side layout only."""
    import ml_dtypes
    bf = ml_dtypes.bfloat16
    f8 = ml_dtypes.float8_e4m3
    local_feats = np.ascontiguousarray(local_feats, dtype=np.float32)
    binary_feats = np.ascontiguousarray(binary_feats, dtype=np.float32)
    W1 = np.ascontiguousarray(W1, dtype=np.float32)
    b1 = np.ascontiguousarray(b1, dtype=np.float32).reshape(1, H)
    W2 = np.ascontiguousarray(W2, dtype=np.float32).reshape(H, 1)
    b2 = np.ascontiguousarray(b2, dtype=np.float32).reshape(1, 1)

    # IND: rows 0..99 = [r==j]+[r==i]; row 100 = ones (b1 row); DR layout
    cols = np.arange(NN2)
    ind2 = np.zeros((N + 1, NN2), dtype=np.float32)
    np.add.at(ind2, (cols % N, cols), 1.0)
    np.add.at(ind2, (cols // N, cols), 1.0)
    ind2[N, :] = 1.0
    ind128 = np.concatenate(
        [ind2, np.zeros((128 - (N + 1), NN2), np.float32)], axis=0)
    ind_dr = np.ascontiguousarray(
        ind128.reshape(2, K64, NN2).transpose(1, 0, 2)).astype(f8)

    cc300 = np.concatenate([b1, W1[H:] * WSCALE], axis=0)    # [12, 300]
    cconst = np.zeros((1 + BIN, 3, 128), dtype=np.float32)
    for kt, (h0, hh) in enumerate(H_T):
        cconst[:, kt, 0:hh] = cc300[:, h0:h0 + hh]
    cconst = cconst.astype(f8)

    in_maps = []
    for c in range(NCORES):
        sl = slice(c * BPC, c * BPC + BPC)
        binT = np.zeros((BPC, 27, NN2), dtype=np.float32)
        binT[:, 0:BIN] = binary_feats[sl].transpose(0, 3, 1, 2).reshape(
            BPC, BIN, NN2) / WSCALE
        binT = binT.astype(f8)
        mc = np.zeros((128, 2103), dtype=np.float32)
        localT = local_feats[sl].transpose(0, 2, 1)          # [BPC, H, N]
        for kt, (k0, kk) in enumerate(H_T):
            mc[0:kk, kt * H:(kt + 1) * H] = W1[k0:k0 + kk, :H]
            for b in range(BPC):
                mc[0:kk, 900 + (b * 3 + kt) * N:900 + (b * 3 + kt + 1) * N] = \
                    localT[b, k0:k0 + kk, :]
            mc[0:kk, 2100 + kt] = W2[k0:k0 + kk, 0]
        for b in range(BPC):
            mc[0:N, 1500 + b * H:1500 + (b + 1) * H] = \
                local_feats[sl][b].reshape(N, H)
        in_maps.append({
            "MC": mc.astype(bf),
            "IND": ind_dr,
            "BINF": binT,
            "Cconst": cconst,
            "b2": b2,
        })
    return in_maps


def _run(in_maps, trace=False):
    from concourse.bass_utils import run_bass_kernel_spmd
    if "nc" not in _CACHE:
        _CACHE["nc"] = _build_nc()
    nc = _CACHE["nc"]
    _CACHE["last_nc"] = nc
    res = run_bass_kernel_spmd(nc, in_maps, core_ids=list(range(NCORES)),
                               trace=trace)
    return res


def kernel(local_feats, binary_feats, sparse_idx, W1, b1, W2, b2):
    local_feats = np.ascontiguousarray(local_feats, dtype=np.float32)
    in_maps = _prep_inputs(local_feats, binary_feats, W1, b1, W2, b2)
    res = _run(in_maps)
    G = np.zeros((B, N, H), dtype=np.float32)
    for c in range(NCORES):
        G[c * BPC:(c + 1) * BPC] = np.asarray(
            res.results[c]["G"], dtype=np.float32).reshape(BPC, N, H)
    sparse_idx = np.asarray(sparse_idx)
    bb = sparse_idx[:, 0].astype(np.int64)
    ii = sparse_idx[:, 1].astype(np.int64)
    jj = sparse_idx[:, 2].astype(np.int64)
    lp = local_feats[bb, ii] + local_feats[bb, jj]
    gp = G[bb, ii] + G[bb, jj]
    return (lp, gp)


# revision 56
# speedup vs baseline: 1.0955x; 1.0652x over previous
"""Trainium2 Bass kernel for nn_Attention_14370960572643 (gnn_message_passing).

Math (per batch b):
  local_pair[b,i,j,:] = local[b,i,:] + local[b,j,:]
  att  = relu(concat(local_pair, binary) @ W1 + b1)        [B,N,N,H]
  score = sigmoid(att @ W2 + b2)                            [B,N,N,1]
  G[b,i,:] = sum_j local[b,j,:] * score[b,i,j]              [B,N,H]
  outputs (E sparse pairs): lp[e] = local[bb,ii]+local[bb,jj]
                            gp[e] = G[bb,ii]+G[bb,jj]

v3 structure:
  * Device computes ONLY att -> score -> G.  The sparse outputs lp/gp are
    pure index-gathers: lp needs only the input local_feats, gp needs only
    the tiny G [B,N,H]; both are assembled host-side after the run.  This
    removes the one-hot gather matmuls, their PSUM->SBUF copies, the oh
    DMA-in and the big lp/gp DMA-outs entirely.
  * att collapses to one K=112 fp8 DoubleRow matmul per (chunk, h-tile):
    contraction rows = 100 P rows (P = local @ W1[:H]) + 1 ones row (b1)
    + 11 W1b rows.  The moving operand packs BOTH pair indicators into the
    identity rows (rhs[r, col] = [r==j(col)] + [r==i(col)]).  The indicator
    part (K 0..100) is batch-independent: ONE SBUF tile holds it, loaded
    once; only the 11 binary rows (slab1 partitions 37..47) are re-DMAed
    between batches.
  * PSUM layout amortizes drain overhead: per 500-col chunk kt0/kt1 land in
    one [128,2,512] f32 tile (one [128,2,500] relu-drain, free=1000); the
    44-row kt2 tiles of 4 consecutive chunks pack into one [128,2,512] bank
    pair at partition offsets 0/44 (one [88,2,500] drain per quad).  50
    drains / 50k free-elements per core, greedily balanced ACT vs DVE.
  * score matmuls are out-free=1 (out = psc[0:100, i:i+1], lhsT = attc
    column block, rhs = W2 tile): ~0 engine cost.  sigmoid -> scT -> G.
"""

import numpy as np

B, N, H, BIN = 16, 100, 300, 11
NN2 = N * N                  # 10000 pair columns per batch
NCORES = 8
BPC = B // NCORES            # batches per core
CH_I = 5                     # i values per chunk
CH = CH_I * N                # 500 pair columns per chunk
NCH = N // CH_I              # 20 chunks per batch
H_T = [(0, 128), (128, 128), (256, 44)]     # h tiles
K112 = N + 1 + BIN           # 112 contraction rows
K64 = 64                     # DoubleRow slab partitions (112 padded to 128)
WSCALE = 16.0                # W1b x16 in C, binary /16 in rhs (fp8 range)

_CACHE = {}


def _build_nc():
    import concourse.bass as bass
    import concourse.mybir as mybir
    import concourse.tile as tile
    from concourse import bacc

    dt = mybir.dt
    f32 = dt.float32
    bf16 = dt.bfloat16
    fp8t = dt.float8e4

    nc = bacc.Bacc("TRN2", target_bir_lowering=False, debug=False,
                   num_devices=NCORES)

    # ---- dram parameters (per-core shards) ----
    # mega-const: W1a (3x300) | localT (6x100) | lnat (2x300) | W2c (3x1)
    mc_d = nc.dram_tensor("MC", [128, 2103], bf16, kind="ExternalInput").ap()
    ind_d = nc.dram_tensor("IND", [K64, 2, NN2], fp8t,
                           kind="ExternalInput").ap()
    # binary rows + zero-pad rows (slab1 partitions 37..63) in one block
    bin_d = nc.dram_tensor("BINF", [BPC, 27, NN2], fp8t,
                           kind="ExternalInput").ap()
    cconst_d = nc.dram_tensor("Cconst", [1 + BIN, 3, 128], fp8t,
                              kind="ExternalInput").ap()
    b2_d = nc.dram_tensor("b2", [1, 1], f32, kind="ExternalInput").ap()
    g_d = nc.dram_tensor("G", [BPC * N, H], bf16, kind="ExternalOutput").ap()

    Relu = mybir.ActivationFunctionType.Relu
    Sigmoid = mybir.ActivationFunctionType.Sigmoid
    DR = mybir.MatmulPerfMode.DoubleRow

    with tile.TileContext(nc) as tc:
        with (
            tc.tile_pool(name="const", bufs=1) as cpool,
            tc.tile_pool(name="attca", bufs=12) as attap,
            tc.tile_pool(name="attcs", bufs=6) as attsp,
            tc.tile_pool(name="paA", bufs=2, space="PSUM") as paA_pool,
            tc.tile_pool(name="paS", bufs=1, space="PSUM") as paS_pool,
            tc.tile_pool(name="ppg", bufs=1, space="PSUM") as pg_pool,
            tc.tile_pool(name="psc", bufs=1, space="PSUM") as psc_pool,
        ):
            # ---------- SBUF constants ----------
            mc = cpool.tile([128, 2103], bf16, tag="mc", name="mc")
            nc.sync.dma_start(out=mc[:, 0:1500], in_=mc_d[:, 0:1500])
            W1a_sb = [mc[0:kk, kt * H:(kt + 1) * H]
                      for kt, (k0, kk) in enumerate(H_T)]
            localT_sb = [[mc[0:kk, 900 + (b * 3 + kt) * N:
                             900 + (b * 3 + kt + 1) * N]
                          for kt, (k0, kk) in enumerate(H_T)]
                         for b in range(BPC)]
            lnat_sb = [mc[0:N, 1500 + b * H:1500 + (b + 1) * H]
                       for b in range(BPC)]
            W2c_sb = [mc[0:hh, 2100 + kt:2101 + kt]
                      for kt, (h0, hh) in enumerate(H_T)]
            b2rep = cpool.tile([128, 1], f32, tag="b2rep", name="b2rep")
            # dummy sigmoid+relu at warmup (fed by memset, no DMA dep) pin
            # the act tables before the drain stream starts
            _junk = cpool.tile([1, 2], f32, tag="junk", name="junk")
            nc.vector.memset(_junk[:], 0.0)
            nc.scalar.activation(_junk[:, 0:1], _junk[:, 1:2], Sigmoid)
            nc.scalar.activation(_junk[:, 0:1], _junk[:, 1:2], Relu)
            # PE p-state warmer: touch the PE early so the 2.4GHz ramp
            # (3us of busy history) completes before the chunk stream
            _wsb = cpool.tile([1, 8], bf16, tag="wsb", name="wsb")
            nc.vector.memset(_wsb[:], 0.0)
            _wps = pg_pool.tile([128, 512], f32, tag="pg", name="wps")
            for _i in range(12):
                nc.tensor.matmul(out=_wps[0:1, 0:8], lhsT=_wsb[:, 0:1],
                                 rhs=_wsb[:], start=True, stop=True)

            # per-(batch, 2500-col piece) rhs tiles: DMA deps are tile-
            # granular, so piece-tiles let chunk matmuls wait only their
            # own piece's loads (lazy-loaded pieces never stall the stream)
            NPC = 2500                   # columns per piece tile
            rhs_sb = [[cpool.tile([K64, 2, NPC], fp8t, tag=f"rhs{b}_{p}",
                                  name=f"rhs{b}_{p}")
                       for p in range(NN2 // NPC)] for b in range(BPC)]
            # per-batch stationary C (fp8 DoubleRow layout, one tile per
            # batch: [K64, slab, kt, 128], kt2 zero-padded past col 44)
            C_sb = []
            scT_sb, g16_sb = [], []
            for b in range(BPC):
                C_sb.append(cpool.tile([K64, 2, 3, 128], fp8t,
                                       tag=f"c{b}", name=f"c{b}"))
                scT_sb.append(cpool.tile([N, N], bf16, tag=f"sct{b}",
                                         name=f"sct{b}"))
                g16_sb.append(cpool.tile([N, H], bf16, tag=f"g16_{b}",
                                         name=f"g16_{b}"))

            def load_ind(b, p):
                # indicator rows of piece p (binary rows ride load_bin)
                sl = slice(p * NPC, (p + 1) * NPC)
                nc.sync.dma_start(out=rhs_sb[b][p][:, 0, :],
                                  in_=ind_d[:, 0, sl])
                nc.sync.dma_start(out=rhs_sb[b][p][0:37, 1, :],
                                  in_=ind_d[0:37, 1, sl])

            def load_bin(b, p):
                # binary + zero-pad rows in one Pool-issued (SWDGE) DMA;
                # the pad rows must be written (uninitialized fp8 can hold
                # NaN, and 0 x NaN = NaN in the PE)
                sl = slice(p * NPC, (p + 1) * NPC)
                nc.gpsimd.dma_start(out=rhs_sb[b][p][37:64, 1, :],
                                    in_=bin_d[b][:, sl])

            def load_cconst(b):
                # whole-tile zero (covers kt2 col pad + slab pads), then
                # rows 100..111 = slab1 partitions 36..47 from dram.
                # cconst rides SP/HWDGE so the Pool queue stays short.
                nc.gpsimd.memset(C_sb[b][:, :, :, :], 0.0)
                nc.sync.dma_start(out=C_sb[b][36:48, 1, :, :],
                                  in_=cconst_d[:, :, :])

            def p_stage(b):
                # P-stages ride startup-idle paA slots so the two batches'
                # stages run in parallel instead of chaining on one bank
                psm3 = paA_pool.tile([128, 2, 512], f32, tag="a",
                                     name=f"psp{b}")
                ps = psm3[0:N, 0, 0:H]
                for kt in range(3):
                    nc.tensor.matmul(out=ps[:], lhsT=localT_sb[b][kt][:],
                                     rhs=W1a_sb[kt][:],
                                     start=(kt == 0), stop=(kt == 2))
                ps2 = psm3[0:N, 0, 0:256].rearrange("p (t c) -> p t c", t=2)
                nc.vector.tensor_copy(out=C_sb[b][0:64, 0, 0:2, 0:128],
                                      in_=ps2[0:64, :, :])
                nc.vector.tensor_copy(out=C_sb[b][0:64, 0, 2, 0:44],
                                      in_=psm3[0:64, 0, 256:300])
                nc.scalar.copy(out=C_sb[b][0:36, 1, 0:2, 0:128],
                               in_=ps2[64:100, :, :])
                nc.scalar.copy(out=C_sb[b][0:36, 1, 2, 0:44],
                               in_=psm3[64:100, 0, 256:300])

            # ---- engine-balanced drain assignment ----
            # greedy: assign each drain to the engine with less accumulated
            # time.  ACT: 0.8333 ns/elem + 185; DVE: 1.0417 ns/elem + 125.
            acc = {"act": 2200.0, "dve": 1500.0}  # bias: ACT sigmoids etc.

            def drain(out_ap, in_ap, nfree):
                t_act = nfree * 0.8333 + 185.0
                t_dve = nfree * 1.0417 + 125.0
                if acc["act"] + t_act <= acc["dve"] + t_dve:
                    acc["act"] += t_act
                    nc.scalar.activation(out_ap, in_ap, Relu)
                else:
                    acc["dve"] += t_dve
                    nc.vector.tensor_scalar_max(out=out_ap, in0=in_ap,
                                                scalar1=0.0)

            def emit_score_group(b, psc, attca, attcs, c, s):
                i = c * CH_I + s
                nc.tensor.matmul(
                    out=psc[0:N, b, i:i + 1],
                    lhsT=attca[0:128, 0, s * N:(s + 1) * N],
                    rhs=W2c_sb[0][:], start=True, stop=False)
                nc.tensor.matmul(
                    out=psc[0:N, b, i:i + 1],
                    lhsT=attca[0:128, 1, s * N:(s + 1) * N],
                    rhs=W2c_sb[1][:], start=False, stop=False)
                nc.tensor.matmul(
                    out=psc[0:N, b, i:i + 1],
                    lhsT=attcs[0:44, b, s * N:(s + 1) * N],
                    rhs=W2c_sb[2][:], start=False, stop=True)

            SIG_CUTS = [(0, 64), (64, N)]

            def emit_sig_g(b, psc, piece):
                """sigmoid + G matmul + g16 copy + out-DMA for an i-range;
                pieces fire as scores accumulate so only the tiny last
                piece sits on the tail."""
                i0, i1 = SIG_CUTS[piece]
                nc.scalar.activation(scT_sb[b][:, i0:i1],
                                     psc[0:N, b, i0:i1], Sigmoid,
                                     bias=b2rep[0:N, :])
                psm = pg_pool.tile([128, 512], f32, tag="pg",
                                   name=f"psg{b}_{piece}")
                nc.tensor.matmul(out=psm[0:i1 - i0, 0:H],
                                 lhsT=scT_sb[b][:, i0:i1],
                                 rhs=lnat_sb[b][:], start=True, stop=True)
                nc.vector.tensor_copy(out=g16_sb[b][i0:i1, :],
                                      in_=psm[0:i1 - i0, 0:H])
                nc.gpsimd.dma_start(
                    out=g_d[b * N + i0:b * N + i1, :],
                    in_=g16_sb[b][i0:i1, :])

            # ------------- interleaved two-stream schedule -------------
            # startup loads.  HWDGE order: MC piece 0 (P-stage), zero-pad
            # rows, first IND pieces of both batches, cconst, the rest.
            # Small loads ride Pool SWDGE.
            load_bin(0, 0)
            load_bin(1, 0)
            load_cconst(0)
            load_cconst(1)
            nc.sync.dma_start(out=b2rep[:],
                              in_=b2_d[0:1, :].to_broadcast([128, 1]))
            load_ind(0, 0)
            load_ind(1, 0)
            # lnat / W2c piece of the mega-const (needed once scores start)
            nc.sync.dma_start(out=mc[:, 1500:2103], in_=mc_d[:, 1500:2103])
            p_stage(0)
            p_stage(1)
            # remaining rhs pieces load lazily, ~5 chunk-levels ahead
            IND_PIECES = {1: 1, 6: 2, 11: 3}

            psc_t = psc_pool.tile([128, 2, 128], f32, tag="sc", name="sc")
            psc = [psc_t for b in range(BPC)]
            attca_t = {}
            squeue = []          # (b, c, s) score groups not yet emitted
            emitted = 0
            sig_done = [0, 0]

            attcs_t = {}

            def emit_scores(upto):
                nonlocal emitted
                while squeue and emitted < upto:
                    bb_, cc, s = squeue.pop(0)
                    emit_score_group(bb_, psc[bb_], attca_t[(bb_, cc)],
                                     attcs_t[cc], cc, s)
                    emitted += 1
                    # fire the mid-stream sigmoid piece when it accumulates
                    # (the final piece runs in the epilogue, after ALL
                    # score matmuls, to avoid PE head-of-line blocking)
                    i_done = cc * CH_I + s
                    if (sig_done[bb_] == 0 and
                            i_done == SIG_CUTS[0][1] - 1):
                        emit_sig_g(bb_, psc[bb_], 0)
                        sig_done[bb_] = 1

            for c in range(NCH):
                c0 = c * CH
                if c in IND_PIECES:
                    p = IND_PIECES[c]
                    load_ind(0, p)
                    load_ind(1, p)
                    load_bin(0, p)
                    load_bin(1, p)
                paS = paS_pool.tile([128, 2, 512], f32, tag="s",
                                    name=f"paS{c}")
                pc0 = c0 - (c0 // NPC) * NPC
                rhs_p = [rhs_sb[b][c0 // NPC] for b in range(BPC)]
                for b in range(BPC):
                    paA = paA_pool.tile([128, 2, 512], f32, tag="a",
                                        name=f"paA{b}_{c}")
                    for kt in range(2):
                        nc.tensor.matmul(
                            out=paA[0:128, kt, 0:CH],
                            lhsT=C_sb[b][:, :, kt, 0:128],
                            rhs=rhs_p[b][:, :, pc0:pc0 + CH],
                            start=True, stop=True, perf_mode=DR)
                    nc.tensor.matmul(
                        out=paS[0:64, b, 0:CH],
                        lhsT=C_sb[b][:, :, 2, 0:64],
                        rhs=rhs_p[b][:, :, pc0:pc0 + CH],
                        start=True, stop=True, perf_mode=DR)
                    attca = attap.tile([128, 2, CH], bf16, tag="attca",
                                       name=f"attca{b}_{c}")
                    attca_t[(b, c)] = attca
                    drain(attca[:, :, :], paA[:, :, 0:CH], 2 * CH)
                    for s in range(CH_I):
                        squeue.append((b, c, s))
                attcs = attsp.tile([64, 2, CH], bf16, tag="attcs",
                                   name=f"attcs{c}")
                attcs_t[c] = attcs
                drain(attcs[:, :, :], paS[0:64, :, 0:CH], 2 * CH)
                # keep scores ~2 chunk-slots behind the matmul stream
                emit_scores((c - 1) * BPC * CH_I)
            emit_scores(10**9)
            emit_sig_g(0, psc[0], 1)
            emit_sig_g(1, psc[1], 1)

    nc.compile()
    return nc


def _prep_inputs(local_feats, binary_feats, W1, b1, W2, b2):
    """Build per-core in_maps. Host-side layout only."""
    import ml_dtypes
    bf = ml_dtypes.bfloat16
    f8 = ml_dtypes.float8_e4m3
    local_feats = np.ascontiguousarray(local_feats, dtype=np.float32)
    binary_feats = np.ascontiguousarray(binary_feats, dtype=np.float32)
    W1 = np.ascontiguousarray(W1, dtype=np.float32)
    b1 = np.ascontiguousarray(b1, dtype=np.float32).reshape(1, H)
    W2 = np.ascontiguousarray(W2, dtype=np.float32).reshape(H, 1)
    b2 = np.ascontiguousarray(b2, dtype=np.float32).reshape(1, 1)

    # IND: rows 0..99 = [r==j]+[r==i]; row 100 = ones (b1 row); DR layout
    cols = np.arange(NN2)
    ind2 = np.zeros((N + 1, NN2), dtype=np.float32)
    np.add.at(ind2, (cols % N, cols), 1.0)
    np.add.at(ind2, (cols // N, cols), 1.0)
    ind2[N, :] = 1.0
    ind128 = np.concatenate(
        [ind2, np.zeros((128 - (N + 1), NN2), np.float32)], axis=0)
    ind_dr = np.ascontiguousarray(
        ind128.reshape(2, K64, NN2).transpose(1, 0, 2)).astype(f8)

    cc300 = np.concatenate([b1, W1[H:] * WSCALE], axis=0)    # [12, 300]
    cconst = np.zeros((1 + BIN, 3, 128), dtype=np.float32)
    for kt, (h0, hh) in enumerate(H_T):
        cconst[:, kt, 0:hh] = cc300[:, h0:h0 + hh]
    cconst = cconst.astype(f8)

    in_maps = []
    for c in range(NCORES):
        sl = slice(c * BPC, c * BPC + BPC)
        binT = np.zeros((BPC, 27, NN2), dtype=np.float32)
        binT[:, 0:BIN] = binary_feats[sl].transpose(0, 3, 1, 2).reshape(
            BPC, BIN, NN2) / WSCALE
        binT = binT.astype(f8)
        mc = np.zeros((128, 2103), dtype=np.float32)
        localT = local_feats[sl].transpose(0, 2, 1)          # [BPC, H, N]
        for kt, (k0, kk) in enumerate(H_T):
            mc[0:kk, kt * H:(kt + 1) * H] = W1[k0:k0 + kk, :H]
            for b in range(BPC):
                mc[0:kk, 900 + (b * 3 + kt) * N:900 + (b * 3 + kt + 1) * N] = \
                    localT[b, k0:k0 + kk, :]
            mc[0:kk, 2100 + kt] = W2[k0:k0 + kk, 0]
        for b in range(BPC):
            mc[0:N, 1500 + b * H:1500 + (b + 1) * H] = \
                local_feats[sl][b].reshape(N, H)
        in_maps.append({
            "MC": mc.astype(bf),
            "IND": ind_dr,
            "BINF": binT,
            "Cconst": cconst,
            "b2": b2,
        })
    return in_maps


def _run(in_maps, trace=False):
    from concourse.bass_utils import run_bass_kernel_spmd
    if "nc" not in _CACHE:
        _CACHE["nc"] = _build_nc()
    nc = _CACHE["nc"]
    _CACHE["last_nc"] = nc
    res = run_bass_kernel_spmd(nc, in_maps, core_ids=list(range(NCORES)),
                               trace=trace)
    return res


def kernel(local_feats, binary_feats, sparse_idx, W1, b1, W2, b2):
    local_feats = np.ascontiguousarray(local_feats, dtype=np.float32)
    in_maps = _prep_inputs(local_feats, binary_feats, W1, b1, W2, b2)
    res = _run(in_maps)
    G = np.zeros((B, N, H), dtype=np.float32)
    for c in range(NCORES):
        G[c * BPC:(c + 1) * BPC] = np.asarray(
            res.results[c]["G"], dtype=np.float32).reshape(BPC, N, H)
    sparse_idx = np.asarray(sparse_idx)
    bb = sparse_idx[:, 0].astype(np.int64)
    ii = sparse_idx[:, 1].astype(np.int64)
    jj = sparse_idx[:, 2].astype(np.int64)
    lp = local_feats[bb, ii] + local_feats[bb, jj]
    gp = G[bb, ii] + G[bb, jj]
    return (lp, gp)


# revision 60
# speedup vs baseline: 1.0959x; 1.0003x over previous
"""Trainium2 Bass kernel for nn_Attention_14370960572643 (gnn_message_passing).

Math (per batch b):
  local_pair[b,i,j,:] = local[b,i,:] + local[b,j,:]
  att  = relu(concat(local_pair, binary) @ W1 + b1)        [B,N,N,H]
  score = sigmoid(att @ W2 + b2)                            [B,N,N,1]
  G[b,i,:] = sum_j local[b,j,:] * score[b,i,j]              [B,N,H]
  outputs (E sparse pairs): lp[e] = local[bb,ii]+local[bb,jj]
                            gp[e] = G[bb,ii]+G[bb,jj]

v3 structure:
  * Device computes ONLY att -> score -> G.  The sparse outputs lp/gp are
    pure index-gathers: lp needs only the input local_feats, gp needs only
    the tiny G [B,N,H]; both are assembled host-side after the run.  This
    removes the one-hot gather matmuls, their PSUM->SBUF copies, the oh
    DMA-in and the big lp/gp DMA-outs entirely.
  * att collapses to one K=112 fp8 DoubleRow matmul per (chunk, h-tile):
    contraction rows = 100 P rows (P = local @ W1[:H]) + 1 ones row (b1)
    + 11 W1b rows.  The moving operand packs BOTH pair indicators into the
    identity rows (rhs[r, col] = [r==j(col)] + [r==i(col)]).  The indicator
    part (K 0..100) is batch-independent: ONE SBUF tile holds it, loaded
    once; only the 11 binary rows (slab1 partitions 37..47) are re-DMAed
    between batches.
  * PSUM layout amortizes drain overhead: per 500-col chunk kt0/kt1 land in
    one [128,2,512] f32 tile (one [128,2,500] relu-drain, free=1000); the
    44-row kt2 tiles of 4 consecutive chunks pack into one [128,2,512] bank
    pair at partition offsets 0/44 (one [88,2,500] drain per quad).  50
    drains / 50k free-elements per core, greedily balanced ACT vs DVE.
  * score matmuls are out-free=1 (out = psc[0:100, i:i+1], lhsT = attc
    column block, rhs = W2 tile): ~0 engine cost.  sigmoid -> scT -> G.
"""

import numpy as np

B, N, H, BIN = 16, 100, 300, 11
NN2 = N * N                  # 10000 pair columns per batch
NCORES = 8
BPC = B // NCORES            # batches per core
CH_I = 5                     # i values per chunk
CH = CH_I * N                # 500 pair columns per chunk
NCH = N // CH_I              # 20 chunks per batch
H_T = [(0, 128), (128, 128), (256, 44)]     # h tiles
K112 = N + 1 + BIN           # 112 contraction rows
K64 = 64                     # DoubleRow slab partitions (112 padded to 128)
WSCALE = 16.0                # W1b x16 in C, binary /16 in rhs (fp8 range)

_CACHE = {}


def _build_nc():
    import concourse.bass as bass
    import concourse.mybir as mybir
    import concourse.tile as tile
    from concourse import bacc

    dt = mybir.dt
    f32 = dt.float32
    bf16 = dt.bfloat16
    fp8t = dt.float8e4

    nc = bacc.Bacc("TRN2", target_bir_lowering=False, debug=False,
                   num_devices=NCORES)

    # ---- dram parameters (per-core shards) ----
    # mega-const: W1a (3x300) | localT (6x100) | lnat (2x300) | W2c (3x1)
    mc_d = nc.dram_tensor("MC", [128, 2103], bf16, kind="ExternalInput").ap()
    ind_d = nc.dram_tensor("IND", [K64, 2, NN2], fp8t,
                           kind="ExternalInput").ap()
    # binary rows + zero-pad rows (slab1 partitions 37..63) in one block
    bin_d = nc.dram_tensor("BINF", [BPC, 27, NN2], fp8t,
                           kind="ExternalInput").ap()
    cconst_d = nc.dram_tensor("Cconst", [1 + BIN, 3, 128], fp8t,
                              kind="ExternalInput").ap()
    b2_d = nc.dram_tensor("b2", [1, 1], f32, kind="ExternalInput").ap()
    g_d = nc.dram_tensor("G", [BPC * N, H], bf16, kind="ExternalOutput").ap()

    Relu = mybir.ActivationFunctionType.Relu
    Sigmoid = mybir.ActivationFunctionType.Sigmoid
    DR = mybir.MatmulPerfMode.DoubleRow

    with tile.TileContext(nc) as tc:
        with (
            tc.tile_pool(name="const", bufs=1) as cpool,
            tc.tile_pool(name="attca", bufs=12) as attap,
            tc.tile_pool(name="attcs", bufs=6) as attsp,
            tc.tile_pool(name="paA", bufs=2, space="PSUM") as paA_pool,
            tc.tile_pool(name="paS", bufs=1, space="PSUM") as paS_pool,
            tc.tile_pool(name="ppg", bufs=1, space="PSUM") as pg_pool,
            tc.tile_pool(name="psc", bufs=1, space="PSUM") as psc_pool,
        ):
            # ---------- SBUF constants ----------
            mc = cpool.tile([128, 2103], bf16, tag="mc", name="mc")
            nc.sync.dma_start(out=mc[:, 0:1500], in_=mc_d[:, 0:1500])
            W1a_sb = [mc[0:kk, kt * H:(kt + 1) * H]
                      for kt, (k0, kk) in enumerate(H_T)]
            localT_sb = [[mc[0:kk, 900 + (b * 3 + kt) * N:
                             900 + (b * 3 + kt + 1) * N]
                          for kt, (k0, kk) in enumerate(H_T)]
                         for b in range(BPC)]
            lnat_sb = [mc[0:N, 1500 + b * H:1500 + (b + 1) * H]
                       for b in range(BPC)]
            W2c_sb = [mc[0:hh, 2100 + kt:2101 + kt]
                      for kt, (h0, hh) in enumerate(H_T)]
            b2rep = cpool.tile([128, 1], f32, tag="b2rep", name="b2rep")
            # dummy sigmoid+relu at warmup (fed by memset, no DMA dep) pin
            # the act tables before the drain stream starts
            _junk = cpool.tile([1, 2], f32, tag="junk", name="junk")
            nc.vector.memset(_junk[:], 0.0)
            nc.scalar.activation(_junk[:, 0:1], _junk[:, 1:2], Sigmoid)
            nc.scalar.activation(_junk[:, 0:1], _junk[:, 1:2], Relu)
            # PE p-state warmer: touch the PE early so the 2.4GHz ramp
            # (3us of busy history) completes before the chunk stream
            _wsb = cpool.tile([1, 8], bf16, tag="wsb", name="wsb")
            nc.vector.memset(_wsb[:], 0.0)
            _wps = pg_pool.tile([128, 512], f32, tag="pg", name="wps")
            for _i in range(12):
                nc.tensor.matmul(out=_wps[0:1, 0:8], lhsT=_wsb[:, 0:1],
                                 rhs=_wsb[:], start=True, stop=True)

            # per-(batch, 2500-col piece) rhs tiles: DMA deps are tile-
            # granular, so piece-tiles let chunk matmuls wait only their
            # own piece's loads (lazy-loaded pieces never stall the stream)
            NPC = 2500                   # columns per piece tile
            rhs_sb = [[cpool.tile([K64, 2, NPC], fp8t, tag=f"rhs{b}_{p}",
                                  name=f"rhs{b}_{p}")
                       for p in range(NN2 // NPC)] for b in range(BPC)]
            # per-batch stationary C (fp8 DoubleRow layout, one tile per
            # batch: [K64, slab, kt, 128], kt2 zero-padded past col 44)
            C_sb = []
            scT_sb, g16_sb = [], []
            for b in range(BPC):
                C_sb.append(cpool.tile([K64, 2, 3, 128], fp8t,
                                       tag=f"c{b}", name=f"c{b}"))
                scT_sb.append(cpool.tile([N, N], bf16, tag=f"sct{b}",
                                         name=f"sct{b}"))
                g16_sb.append(cpool.tile([N, H], bf16, tag=f"g16_{b}",
                                         name=f"g16_{b}"))

            def load_ind(b, p):
                # indicator rows of piece p (binary rows ride load_bin)
                sl = slice(p * NPC, (p + 1) * NPC)
                nc.sync.dma_start(out=rhs_sb[b][p][:, 0, :],
                                  in_=ind_d[:, 0, sl])
                nc.sync.dma_start(out=rhs_sb[b][p][0:37, 1, :],
                                  in_=ind_d[0:37, 1, sl])

            def load_bin(b, p, eng=None):
                # binary + zero-pad rows in one DMA (Pool SWDGE for the
                # lazy pieces; piece 0 rides SP so its transfer doesn't
                # preempt the mega-const on the DMA engines).  Pad rows
                # must be written: uninitialized fp8 can hold NaN.
                sl = slice(p * NPC, (p + 1) * NPC)
                (eng or nc.gpsimd).dma_start(out=rhs_sb[b][p][37:64, 1, :],
                                             in_=bin_d[b][:, sl])

            def load_cconst(b):
                # whole-tile zero (covers kt2 col pad + slab pads), then
                # rows 100..111 = slab1 partitions 36..47 from dram.
                # cconst rides SP/HWDGE so the Pool queue stays short.
                nc.gpsimd.memset(C_sb[b][:, :, :, :], 0.0)
                nc.sync.dma_start(out=C_sb[b][36:48, 1, :, :],
                                  in_=cconst_d[:, :, :])

            def p_stage(b):
                # P-stages ride startup-idle paA slots so the two batches'
                # stages run in parallel instead of chaining on one bank
                psm3 = paA_pool.tile([128, 2, 512], f32, tag="a",
                                     name=f"psp{b}")
                ps = psm3[0:N, 0, 0:H]
                for kt in range(3):
                    nc.tensor.matmul(out=ps[:], lhsT=localT_sb[b][kt][:],
                                     rhs=W1a_sb[kt][:],
                                     start=(kt == 0), stop=(kt == 2))
                ps2 = psm3[0:N, 0, 0:256].rearrange("p (t c) -> p t c", t=2)
                nc.vector.tensor_copy(out=C_sb[b][0:64, 0, 0:2, 0:128],
                                      in_=ps2[0:64, :, :])
                nc.vector.tensor_copy(out=C_sb[b][0:64, 0, 2, 0:44],
                                      in_=psm3[0:64, 0, 256:300])
                nc.scalar.copy(out=C_sb[b][0:36, 1, 0:2, 0:128],
                               in_=ps2[64:100, :, :])
                nc.scalar.copy(out=C_sb[b][0:36, 1, 2, 0:44],
                               in_=psm3[64:100, 0, 256:300])

            # ---- engine-balanced drain assignment ----
            # greedy: assign each drain to the engine with less accumulated
            # time.  ACT: 0.8333 ns/elem + 185; DVE: 1.0417 ns/elem + 125.
            # bias by each engine's non-drain duties: ACT sigmoids +
            # scalar C-copies, DVE g16-copies + vector C-copies
            acc = {"act": 2100.0, "dve": 3700.0}

            def drain(out_ap, in_ap, nfree):
                t_act = nfree * 0.8333 + 185.0
                t_dve = nfree * 1.0417 + 125.0
                if acc["act"] + t_act <= acc["dve"] + t_dve:
                    acc["act"] += t_act
                    nc.scalar.activation(out_ap, in_ap, Relu)
                else:
                    acc["dve"] += t_dve
                    nc.vector.tensor_scalar_max(out=out_ap, in0=in_ap,
                                                scalar1=0.0)

            def emit_score_group(b, psc, attca, attcs, c, s):
                i = c * CH_I + s
                nc.tensor.matmul(
                    out=psc[0:N, b, i:i + 1],
                    lhsT=attca[0:128, 0, s * N:(s + 1) * N],
                    rhs=W2c_sb[0][:], start=True, stop=False)
                nc.tensor.matmul(
                    out=psc[0:N, b, i:i + 1],
                    lhsT=attca[0:128, 1, s * N:(s + 1) * N],
                    rhs=W2c_sb[1][:], start=False, stop=False)
                nc.tensor.matmul(
                    out=psc[0:N, b, i:i + 1],
                    lhsT=attcs[0:44, b, s * N:(s + 1) * N],
                    rhs=W2c_sb[2][:], start=False, stop=True)

            SIG_CUTS = [(0, 64), (64, N)]

            def emit_sig_g(b, psc, piece):
                """sigmoid + G matmul + g16 copy + out-DMA for an i-range;
                pieces fire as scores accumulate so only the tiny last
                piece sits on the tail."""
                i0, i1 = SIG_CUTS[piece]
                nc.scalar.activation(scT_sb[b][:, i0:i1],
                                     psc[0:N, b, i0:i1], Sigmoid,
                                     bias=b2rep[0:N, :])
                psm = pg_pool.tile([128, 512], f32, tag="pg",
                                   name=f"psg{b}_{piece}")
                nc.tensor.matmul(out=psm[0:i1 - i0, 0:H],
                                 lhsT=scT_sb[b][:, i0:i1],
                                 rhs=lnat_sb[b][:], start=True, stop=True)
                nc.vector.tensor_copy(out=g16_sb[b][i0:i1, :],
                                      in_=psm[0:i1 - i0, 0:H])
                # final pieces launch on different queues so the two
                # batches' tail DMAs don't serialize on one engine
                eng = nc.gpsimd if (piece == 0 or b == 0) else nc.sync
                eng.dma_start(out=g_d[b * N + i0:b * N + i1, :],
                              in_=g16_sb[b][i0:i1, :])

            # ------------- interleaved two-stream schedule -------------
            # startup loads.  HWDGE order: MC piece 0 (P-stage), zero-pad
            # rows, first IND pieces of both batches, cconst, the rest.
            # Small loads ride Pool SWDGE.
            load_bin(0, 0, nc.sync)
            load_bin(1, 0, nc.sync)
            load_cconst(0)
            load_cconst(1)
            nc.sync.dma_start(out=b2rep[:],
                              in_=b2_d[0:1, :].to_broadcast([128, 1]))
            load_ind(0, 0)
            load_ind(1, 0)
            # lnat / W2c piece of the mega-const (needed once scores start)
            nc.sync.dma_start(out=mc[:, 1500:2103], in_=mc_d[:, 1500:2103])
            p_stage(0)
            p_stage(1)
            # remaining rhs pieces load lazily, ~5 chunk-levels ahead
            IND_PIECES = {1: 1, 6: 2, 11: 3}

            psc_t = psc_pool.tile([128, 2, 128], f32, tag="sc", name="sc")
            psc = [psc_t for b in range(BPC)]
            attca_t = {}
            squeue = []          # (b, c, s) score groups not yet emitted
            emitted = 0
            sig_done = [0, 0]

            attcs_t = {}

            def emit_scores(upto):
                nonlocal emitted
                while squeue and emitted < upto:
                    bb_, cc, s = squeue.pop(0)
                    emit_score_group(bb_, psc[bb_], attca_t[(bb_, cc)],
                                     attcs_t[cc], cc, s)
                    emitted += 1
                    # fire the mid-stream sigmoid piece when it accumulates
                    # (the final piece runs in the epilogue, after ALL
                    # score matmuls, to avoid PE head-of-line blocking)
                    i_done = cc * CH_I + s
                    if (sig_done[bb_] == 0 and
                            i_done == SIG_CUTS[0][1] - 1):
                        emit_sig_g(bb_, psc[bb_], 0)
                        sig_done[bb_] = 1

            for c in range(NCH):
                c0 = c * CH
                if c in IND_PIECES:
                    p = IND_PIECES[c]
                    load_ind(0, p)
                    load_ind(1, p)
                    load_bin(0, p)
                    load_bin(1, p)
                paS = paS_pool.tile([128, 2, 512], f32, tag="s",
                                    name=f"paS{c}")
                pc0 = c0 - (c0 // NPC) * NPC
                rhs_p = [rhs_sb[b][c0 // NPC] for b in range(BPC)]
                for b in range(BPC):
                    paA = paA_pool.tile([128, 2, 512], f32, tag="a",
                                        name=f"paA{b}_{c}")
                    for kt in range(2):
                        nc.tensor.matmul(
                            out=paA[0:128, kt, 0:CH],
                            lhsT=C_sb[b][:, :, kt, 0:128],
                            rhs=rhs_p[b][:, :, pc0:pc0 + CH],
                            start=True, stop=True, perf_mode=DR)
                    nc.tensor.matmul(
                        out=paS[0:64, b, 0:CH],
                        lhsT=C_sb[b][:, :, 2, 0:64],
                        rhs=rhs_p[b][:, :, pc0:pc0 + CH],
                        start=True, stop=True, perf_mode=DR)
                    attca = attap.tile([128, 2, CH], bf16, tag="attca",
                                       name=f"attca{b}_{c}")
                    attca_t[(b, c)] = attca
                    drain(attca[:, :, :], paA[:, :, 0:CH], 2 * CH)
                    for s in range(CH_I):
                        squeue.append((b, c, s))
                attcs = attsp.tile([64, 2, CH], bf16, tag="attcs",
                                   name=f"attcs{c}")
                attcs_t[c] = attcs
                drain(attcs[:, :, :], paS[0:64, :, 0:CH], 2 * CH)
                # keep scores ~2 chunk-slots behind the matmul stream
                emit_scores((c - 1) * BPC * CH_I)
            emit_scores(10**9)
            emit_sig_g(0, psc[0], 1)
            emit_sig_g(1, psc[1], 1)

    nc.compile()
    return nc


def _prep_inputs(local_feats, binary_feats, W1, b1, W2, b2):
    """Build per-core in_maps. Host-side layout only."""
    import ml_dtypes
    bf = ml_dtypes.bfloat16
    f8 = ml_dtypes.float8_e4m3
    local_feats = np.ascontiguousarray(local_feats, dtype=np.float32)
    binary_feats = np.ascontiguousarray(binary_feats, dtype=np.float32)
    W1 = np.ascontiguousarray(W1, dtype=np.float32)
    b1 = np.ascontiguousarray(b1, dtype=np.float32).reshape(1, H)
    W2 = np.ascontiguousarray(W2, dtype=np.float32).reshape(H, 1)
    b2 = np.ascontiguousarray(b2, dtype=np.float32).reshape(1, 1)

    # IND: rows 0..99 = [r==j]+[r==i]; row 100 = ones (b1 row); DR layout
    cols = np.arange(NN2)
    ind2 = np.zeros((N + 1, NN2), dtype=np.float32)
    np.add.at(ind2, (cols % N, cols), 1.0)
    np.add.at(ind2, (cols // N, cols), 1.0)
    ind2[N, :] = 1.0
    ind128 = np.concatenate(
        [ind2, np.zeros((128 - (N + 1), NN2), np.float32)], axis=0)
    ind_dr = np.ascontiguousarray(
        ind128.reshape(2, K64, NN2).transpose(1, 0, 2)).astype(f8)

    cc300 = np.concatenate([b1, W1[H:] * WSCALE], axis=0)    # [12, 300]
    cconst = np.zeros((1 + BIN, 3, 128), dtype=np.float32)
    for kt, (h0, hh) in enumerate(H_T):
        cconst[:, kt, 0:hh] = cc300[:, h0:h0 + hh]
    cconst = cconst.astype(f8)

    in_maps = []
    for c in range(NCORES):
        sl = slice(c * BPC, c * BPC + BPC)
        binT = np.zeros((BPC, 27, NN2), dtype=np.float32)
        binT[:, 0:BIN] = binary_feats[sl].transpose(0, 3, 1, 2).reshape(
            BPC, BIN, NN2) / WSCALE
        binT = binT.astype(f8)
        mc = np.zeros((128, 2103), dtype=np.float32)
        localT = local_feats[sl].transpose(0, 2, 1)          # [BPC, H, N]
        for kt, (k0, kk) in enumerate(H_T):
            mc[0:kk, kt * H:(kt + 1) * H] = W1[k0:k0 + kk, :H]
            for b in range(BPC):
                mc[0:kk, 900 + (b * 3 + kt) * N:900 + (b * 3 + kt + 1) * N] = \
                    localT[b, k0:k0 + kk, :]
            mc[0:kk, 2100 + kt] = W2[k0:k0 + kk, 0]
        for b in range(BPC):
            mc[0:N, 1500 + b * H:1500 + (b + 1) * H] = \
                local_feats[sl][b].reshape(N, H)
        in_maps.append({
            "MC": mc.astype(bf),
            "IND": ind_dr,
            "BINF": binT,
            "Cconst": cconst,
            "b2": b2,
        })
    return in_maps


def _run(in_maps, trace=False):
    from concourse.bass_utils import run_bass_kernel_spmd
    if "nc" not in _CACHE:
        _CACHE["nc"] = _build_nc()
    nc = _CACHE["nc"]
    _CACHE["last_nc"] = nc
    res = run_bass_kernel_spmd(nc, in_maps, core_ids=list(range(NCORES)),
                               trace=trace)
    return res


def kernel(local_feats, binary_feats, sparse_idx, W1, b1, W2, b2):
    local_feats = np.ascontiguousarray(local_feats, dtype=np.float32)
    in_maps = _prep_inputs(local_feats, binary_feats, W1, b1, W2, b2)
    res = _run(in_maps)
    G = np.zeros((B, N, H), dtype=np.float32)
    for c in range(NCORES):
        G[c * BPC:(c + 1) * BPC] = np.asarray(
            res.results[c]["G"], dtype=np.float32).reshape(BPC, N, H)
    sparse_idx = np.asarray(sparse_idx)
    bb = sparse_idx[:, 0].astype(np.int64)
    ii = sparse_idx[:, 1].astype(np.int64)
    jj = sparse_idx[:, 2].astype(np.int64)
    lp = local_feats[bb, ii] + local_feats[bb, jj]
    gp = G[bb, ii] + G[bb, jj]
    return (lp, gp)


# revision 61
# speedup vs baseline: 1.1003x; 1.0040x over previous
"""Trainium2 Bass kernel for nn_Attention_14370960572643 (gnn_message_passing).

Math (per batch b):
  local_pair[b,i,j,:] = local[b,i,:] + local[b,j,:]
  att  = relu(concat(local_pair, binary) @ W1 + b1)        [B,N,N,H]
  score = sigmoid(att @ W2 + b2)                            [B,N,N,1]
  G[b,i,:] = sum_j local[b,j,:] * score[b,i,j]              [B,N,H]
  outputs (E sparse pairs): lp[e] = local[bb,ii]+local[bb,jj]
                            gp[e] = G[bb,ii]+G[bb,jj]

v3 structure:
  * Device computes ONLY att -> score -> G.  The sparse outputs lp/gp are
    pure index-gathers: lp needs only the input local_feats, gp needs only
    the tiny G [B,N,H]; both are assembled host-side after the run.  This
    removes the one-hot gather matmuls, their PSUM->SBUF copies, the oh
    DMA-in and the big lp/gp DMA-outs entirely.
  * att collapses to one K=112 fp8 DoubleRow matmul per (chunk, h-tile):
    contraction rows = 100 P rows (P = local @ W1[:H]) + 1 ones row (b1)
    + 11 W1b rows.  The moving operand packs BOTH pair indicators into the
    identity rows (rhs[r, col] = [r==j(col)] + [r==i(col)]).  The indicator
    part (K 0..100) is batch-independent: ONE SBUF tile holds it, loaded
    once; only the 11 binary rows (slab1 partitions 37..47) are re-DMAed
    between batches.
  * PSUM layout amortizes drain overhead: per 500-col chunk kt0/kt1 land in
    one [128,2,512] f32 tile (one [128,2,500] relu-drain, free=1000); the
    44-row kt2 tiles of 4 consecutive chunks pack into one [128,2,512] bank
    pair at partition offsets 0/44 (one [88,2,500] drain per quad).  50
    drains / 50k free-elements per core, greedily balanced ACT vs DVE.
  * score matmuls are out-free=1 (out = psc[0:100, i:i+1], lhsT = attc
    column block, rhs = W2 tile): ~0 engine cost.  sigmoid -> scT -> G.
"""

import numpy as np

B, N, H, BIN = 16, 100, 300, 11
NN2 = N * N                  # 10000 pair columns per batch
NCORES = 8
BPC = B // NCORES            # batches per core
CH_I = 5                     # i values per chunk
CH = CH_I * N                # 500 pair columns per chunk
NCH = N // CH_I              # 20 chunks per batch
H_T = [(0, 128), (128, 128), (256, 44)]     # h tiles
K112 = N + 1 + BIN           # 112 contraction rows
K64 = 64                     # DoubleRow slab partitions (112 padded to 128)
WSCALE = 16.0                # W1b x16 in C, binary /16 in rhs (fp8 range)

_CACHE = {}


def _build_nc():
    import concourse.bass as bass
    import concourse.mybir as mybir
    import concourse.tile as tile
    from concourse import bacc

    dt = mybir.dt
    f32 = dt.float32
    bf16 = dt.bfloat16
    fp8t = dt.float8e4

    nc = bacc.Bacc("TRN2", target_bir_lowering=False, debug=False,
                   num_devices=NCORES)

    # ---- dram parameters (per-core shards) ----
    # mega-const: W1a (3x300) | localT (6x100) | lnat (2x300) | W2c (3x1)
    mc_d = nc.dram_tensor("MC", [128, 2103], bf16, kind="ExternalInput").ap()
    ind_d = nc.dram_tensor("IND", [K64, 2, NN2], fp8t,
                           kind="ExternalInput").ap()
    # binary rows + zero-pad rows (slab1 partitions 37..63) in one block
    bin_d = nc.dram_tensor("BINF", [BPC, 27, NN2], fp8t,
                           kind="ExternalInput").ap()
    cconst_d = nc.dram_tensor("Cconst", [1 + BIN, 3, 128], fp8t,
                              kind="ExternalInput").ap()
    b2_d = nc.dram_tensor("b2", [1, 1], f32, kind="ExternalInput").ap()
    g_d = nc.dram_tensor("G", [BPC * N, H], bf16, kind="ExternalOutput").ap()

    Relu = mybir.ActivationFunctionType.Relu
    Sigmoid = mybir.ActivationFunctionType.Sigmoid
    DR = mybir.MatmulPerfMode.DoubleRow

    with tile.TileContext(nc) as tc:
        with (
            tc.tile_pool(name="const", bufs=1) as cpool,
            tc.tile_pool(name="attca", bufs=12) as attap,
            tc.tile_pool(name="attcs", bufs=6) as attsp,
            tc.tile_pool(name="paA", bufs=2, space="PSUM") as paA_pool,
            tc.tile_pool(name="paS", bufs=1, space="PSUM") as paS_pool,
            tc.tile_pool(name="ppg", bufs=1, space="PSUM") as pg_pool,
            tc.tile_pool(name="psc", bufs=1, space="PSUM") as psc_pool,
        ):
            # ---------- SBUF constants ----------
            mc = cpool.tile([128, 2103], bf16, tag="mc", name="mc")
            nc.sync.dma_start(out=mc[:, 0:1500], in_=mc_d[:, 0:1500])
            W1a_sb = [mc[0:kk, kt * H:(kt + 1) * H]
                      for kt, (k0, kk) in enumerate(H_T)]
            localT_sb = [[mc[0:kk, 900 + (b * 3 + kt) * N:
                             900 + (b * 3 + kt + 1) * N]
                          for kt, (k0, kk) in enumerate(H_T)]
                         for b in range(BPC)]
            lnat_sb = [mc[0:N, 1500 + b * H:1500 + (b + 1) * H]
                       for b in range(BPC)]
            W2c_sb = [mc[0:hh, 2100 + kt:2101 + kt]
                      for kt, (h0, hh) in enumerate(H_T)]
            b2rep = cpool.tile([128, 1], f32, tag="b2rep", name="b2rep")
            # dummy sigmoid+relu at warmup (fed by memset, no DMA dep) pin
            # the act tables before the drain stream starts
            _junk = cpool.tile([1, 2], f32, tag="junk", name="junk")
            nc.vector.memset(_junk[:], 0.0)
            nc.scalar.activation(_junk[:, 0:1], _junk[:, 1:2], Sigmoid)
            nc.scalar.activation(_junk[:, 0:1], _junk[:, 1:2], Relu)
            # PE p-state warmer: touch the PE early so the 2.4GHz ramp
            # (3us of busy history) completes before the chunk stream
            _wsb = cpool.tile([1, 8], bf16, tag="wsb", name="wsb")
            nc.vector.memset(_wsb[:], 0.0)
            _wps = pg_pool.tile([128, 512], f32, tag="pg", name="wps")
            for _i in range(12):
                nc.tensor.matmul(out=_wps[0:1, 0:8], lhsT=_wsb[:, 0:1],
                                 rhs=_wsb[:], start=True, stop=True)

            # per-(batch, 2500-col piece) rhs tiles: DMA deps are tile-
            # granular, so piece-tiles let chunk matmuls wait only their
            # own piece's loads (lazy-loaded pieces never stall the stream)
            NPC = 2500                   # columns per piece tile
            rhs_sb = [[cpool.tile([K64, 2, NPC], fp8t, tag=f"rhs{b}_{p}",
                                  name=f"rhs{b}_{p}")
                       for p in range(NN2 // NPC)] for b in range(BPC)]
            # per-batch stationary C (fp8 DoubleRow layout, one tile per
            # batch: [K64, slab, kt, 128], kt2 zero-padded past col 44)
            C_sb = []
            scT_sb, g16_sb = [], []
            for b in range(BPC):
                C_sb.append(cpool.tile([K64, 2, 3, 128], fp8t,
                                       tag=f"c{b}", name=f"c{b}"))
                scT_sb.append(cpool.tile([N, N], bf16, tag=f"sct{b}",
                                         name=f"sct{b}"))
                g16_sb.append(cpool.tile([N, H], bf16, tag=f"g16_{b}",
                                         name=f"g16_{b}"))

            def load_ind(b, p):
                # indicator rows of piece p (binary rows ride load_bin)
                sl = slice(p * NPC, (p + 1) * NPC)
                nc.sync.dma_start(out=rhs_sb[b][p][:, 0, :],
                                  in_=ind_d[:, 0, sl])
                nc.sync.dma_start(out=rhs_sb[b][p][0:37, 1, :],
                                  in_=ind_d[0:37, 1, sl])

            def load_bin(b, p, eng=None):
                # binary + zero-pad rows in one DMA (Pool SWDGE for the
                # lazy pieces; piece 0 rides SP so its transfer doesn't
                # preempt the mega-const on the DMA engines).  Pad rows
                # must be written: uninitialized fp8 can hold NaN.
                sl = slice(p * NPC, (p + 1) * NPC)
                (eng or nc.gpsimd).dma_start(out=rhs_sb[b][p][37:64, 1, :],
                                             in_=bin_d[b][:, sl])

            def load_cconst(b):
                # whole-tile zero (covers kt2 col pad + slab pads), then
                # rows 100..111 = slab1 partitions 36..47 from dram.
                # cconst rides SP/HWDGE so the Pool queue stays short.
                nc.gpsimd.memset(C_sb[b][:, :, :, :], 0.0)
                nc.sync.dma_start(out=C_sb[b][36:48, 1, :, :],
                                  in_=cconst_d[:, :, :])

            def p_stage(b):
                # P-stages ride startup-idle paA slots so the two batches'
                # stages run in parallel instead of chaining on one bank
                psm3 = paA_pool.tile([128, 2, 512], f32, tag="a",
                                     name=f"psp{b}")
                ps = psm3[0:N, 0, 0:H]
                for kt in range(3):
                    nc.tensor.matmul(out=ps[:], lhsT=localT_sb[b][kt][:],
                                     rhs=W1a_sb[kt][:],
                                     start=(kt == 0), stop=(kt == 2))
                ps2 = psm3[0:N, 0, 0:256].rearrange("p (t c) -> p t c", t=2)
                nc.vector.tensor_copy(out=C_sb[b][0:64, 0, 0:2, 0:128],
                                      in_=ps2[0:64, :, :])
                nc.vector.tensor_copy(out=C_sb[b][0:64, 0, 2, 0:44],
                                      in_=psm3[0:64, 0, 256:300])
                nc.scalar.copy(out=C_sb[b][0:36, 1, 0:2, 0:128],
                               in_=ps2[64:100, :, :])
                nc.scalar.copy(out=C_sb[b][0:36, 1, 2, 0:44],
                               in_=psm3[64:100, 0, 256:300])

            # ---- engine-balanced drain assignment ----
            # greedy: assign each drain to the engine with less accumulated
            # time.  ACT: 0.8333 ns/elem + 185; DVE: 1.0417 ns/elem + 125.
            # bias by each engine's non-drain duties: ACT sigmoids +
            # scalar C-copies, DVE g16-copies + vector C-copies
            acc = {"act": 4200.0, "dve": 3500.0}

            def drain(out_ap, in_ap, nfree):
                t_act = nfree * 0.8333 + 185.0
                t_dve = nfree * 1.0417 + 125.0
                if acc["act"] + t_act <= acc["dve"] + t_dve:
                    acc["act"] += t_act
                    nc.scalar.activation(out_ap, in_ap, Relu)
                else:
                    acc["dve"] += t_dve
                    nc.vector.tensor_scalar_max(out=out_ap, in0=in_ap,
                                                scalar1=0.0)

            def emit_score_group(b, psc, attca, attcs, c, s):
                i = c * CH_I + s
                nc.tensor.matmul(
                    out=psc[0:N, b, i:i + 1],
                    lhsT=attca[0:128, 0, s * N:(s + 1) * N],
                    rhs=W2c_sb[0][:], start=True, stop=False)
                nc.tensor.matmul(
                    out=psc[0:N, b, i:i + 1],
                    lhsT=attca[0:128, 1, s * N:(s + 1) * N],
                    rhs=W2c_sb[1][:], start=False, stop=False)
                nc.tensor.matmul(
                    out=psc[0:N, b, i:i + 1],
                    lhsT=attcs[0:44, b, s * N:(s + 1) * N],
                    rhs=W2c_sb[2][:], start=False, stop=True)

            SIG_CUTS = [(0, 64), (64, N)]

            def emit_sig_g(b, psc, piece):
                """sigmoid + G matmul + g16 copy + out-DMA for an i-range;
                pieces fire as scores accumulate so only the tiny last
                piece sits on the tail."""
                i0, i1 = SIG_CUTS[piece]
                nc.scalar.activation(scT_sb[b][:, i0:i1],
                                     psc[0:N, b, i0:i1], Sigmoid,
                                     bias=b2rep[0:N, :])
                psm = pg_pool.tile([128, 512], f32, tag="pg",
                                   name=f"psg{b}_{piece}")
                nc.tensor.matmul(out=psm[0:i1 - i0, 0:H],
                                 lhsT=scT_sb[b][:, i0:i1],
                                 rhs=lnat_sb[b][:], start=True, stop=True)
                nc.vector.tensor_copy(out=g16_sb[b][i0:i1, :],
                                      in_=psm[0:i1 - i0, 0:H])
                # final pieces launch on different queues so the two
                # batches' tail DMAs don't serialize on one engine
                eng = nc.gpsimd if (piece == 0 or b == 0) else nc.sync
                eng.dma_start(out=g_d[b * N + i0:b * N + i1, :],
                              in_=g16_sb[b][i0:i1, :])

            # ------------- interleaved two-stream schedule -------------
            # startup loads.  HWDGE order: MC piece 0 (P-stage), zero-pad
            # rows, first IND pieces of both batches, cconst, the rest.
            # Small loads ride Pool SWDGE.
            load_bin(0, 0)
            load_bin(1, 0)
            load_cconst(0)
            load_cconst(1)
            load_ind(0, 0)
            load_ind(1, 0)
            nc.sync.dma_start(out=b2rep[:],
                              in_=b2_d[0:1, :].to_broadcast([128, 1]))
            # lnat / W2c piece of the mega-const (needed once scores start)
            nc.sync.dma_start(out=mc[:, 1500:2103], in_=mc_d[:, 1500:2103])
            p_stage(0)
            p_stage(1)
            # remaining rhs pieces load lazily, ~5 chunk-levels ahead
            IND_PIECES = {1: 1, 6: 2, 11: 3}

            psc_t = psc_pool.tile([128, 2, 128], f32, tag="sc", name="sc")
            psc = [psc_t for b in range(BPC)]
            attca_t = {}
            squeue = []          # (b, c, s) score groups not yet emitted
            emitted = 0
            sig_done = [0, 0]

            attcs_t = {}

            def emit_scores(upto):
                nonlocal emitted
                while squeue and emitted < upto:
                    bb_, cc, s = squeue.pop(0)
                    emit_score_group(bb_, psc[bb_], attca_t[(bb_, cc)],
                                     attcs_t[cc], cc, s)
                    emitted += 1
                    # fire the mid-stream sigmoid piece when it accumulates
                    # (the final piece runs in the epilogue, after ALL
                    # score matmuls, to avoid PE head-of-line blocking)
                    i_done = cc * CH_I + s
                    if (sig_done[bb_] == 0 and
                            i_done == SIG_CUTS[0][1] - 1):
                        emit_sig_g(bb_, psc[bb_], 0)
                        sig_done[bb_] = 1

            for c in range(NCH):
                c0 = c * CH
                if c in IND_PIECES:
                    p = IND_PIECES[c]
                    load_ind(0, p)
                    load_ind(1, p)
                    load_bin(0, p)
                    load_bin(1, p)
                paS = paS_pool.tile([128, 2, 512], f32, tag="s",
                                    name=f"paS{c}")
                pc0 = c0 - (c0 // NPC) * NPC
                rhs_p = [rhs_sb[b][c0 // NPC] for b in range(BPC)]
                for b in range(BPC):
                    paA = paA_pool.tile([128, 2, 512], f32, tag="a",
                                        name=f"paA{b}_{c}")
                    for kt in range(2):
                        nc.tensor.matmul(
                            out=paA[0:128, kt, 0:CH],
                            lhsT=C_sb[b][:, :, kt, 0:128],
                            rhs=rhs_p[b][:, :, pc0:pc0 + CH],
                            start=True, stop=True, perf_mode=DR)
                    nc.tensor.matmul(
                        out=paS[0:64, b, 0:CH],
                        lhsT=C_sb[b][:, :, 2, 0:64],
                        rhs=rhs_p[b][:, :, pc0:pc0 + CH],
                        start=True, stop=True, perf_mode=DR)
                    attca = attap.tile([128, 2, CH], bf16, tag="attca",
                                       name=f"attca{b}_{c}")
                    attca_t[(b, c)] = attca
                    drain(attca[:, :, :], paA[:, :, 0:CH], 2 * CH)
                    for s in range(CH_I):
                        squeue.append((b, c, s))
                attcs = attsp.tile([64, 2, CH], bf16, tag="attcs",
                                   name=f"attcs{c}")
                attcs_t[c] = attcs
                drain(attcs[:, :, :], paS[0:64, :, 0:CH], 2 * CH)
                # keep scores ~2 chunk-slots behind the matmul stream
                emit_scores((c - 1) * BPC * CH_I)
            emit_scores(10**9)
            emit_sig_g(1, psc[1], 1)
            emit_sig_g(0, psc[0], 1)

    nc.compile()
    return nc


def _prep_inputs(local_feats, binary_feats, W1, b1, W2, b2):
    """Build per-core in_maps. Host-side layout only."""
    import ml_dtypes
    bf = ml_dtypes.bfloat16
    f8 = ml_dtypes.float8_e4m3
    local_feats = np.ascontiguousarray(local_feats, dtype=np.float32)
    binary_feats = np.ascontiguousarray(binary_feats, dtype=np.float32)
    W1 = np.ascontiguousarray(W1, dtype=np.float32)
    b1 = np.ascontiguousarray(b1, dtype=np.float32).reshape(1, H)
    W2 = np.ascontiguousarray(W2, dtype=np.float32).reshape(H, 1)
    b2 = np.ascontiguousarray(b2, dtype=np.float32).reshape(1, 1)

    # IND: rows 0..99 = [r==j]+[r==i]; row 100 = ones (b1 row); DR layout
    cols = np.arange(NN2)
    ind2 = np.zeros((N + 1, NN2), dtype=np.float32)
    np.add.at(ind2, (cols % N, cols), 1.0)
    np.add.at(ind2, (cols // N, cols), 1.0)
    ind2[N, :] = 1.0
    ind128 = np.concatenate(
        [ind2, np.zeros((128 - (N + 1), NN2), np.float32)], axis=0)
    ind_dr = np.ascontiguousarray(
        ind128.reshape(2, K64, NN2).transpose(1, 0, 2)).astype(f8)

    cc300 = np.concatenate([b1, W1[H:] * WSCALE], axis=0)    # [12, 300]
    cconst = np.zeros((1 + BIN, 3, 128), dtype=np.float32)
    for kt, (h0, hh) in enumerate(H_T):
        cconst[:, kt, 0:hh] = cc300[:, h0:h0 + hh]
    cconst = cconst.astype(f8)

    in_maps = []
    for c in range(NCORES):
        sl = slice(c * BPC, c * BPC + BPC)
        binT = np.zeros((BPC, 27, NN2), dtype=np.float32)
        binT[:, 0:BIN] = binary_feats[sl].transpose(0, 3, 1, 2).reshape(
            BPC, BIN, NN2) / WSCALE
        binT = binT.astype(f8)
        mc = np.zeros((128, 2103), dtype=np.float32)
        localT = local_feats[sl].transpose(0, 2, 1)          # [BPC, H, N]
        for kt, (k0, kk) in enumerate(H_T):
            mc[0:kk, kt * H:(kt + 1) * H] = W1[k0:k0 + kk, :H]
            for b in range(BPC):
                mc[0:kk, 900 + (b * 3 + kt) * N:900 + (b * 3 + kt + 1) * N] = \
                    localT[b, k0:k0 + kk, :]
            mc[0:kk, 2100 + kt] = W2[k0:k0 + kk, 0]
        for b in range(BPC):
            mc[0:N, 1500 + b * H:1500 + (b + 1) * H] = \
                local_feats[sl][b].reshape(N, H)
        in_maps.append({
            "MC": mc.astype(bf),
            "IND": ind_dr,
            "BINF": binT,
            "Cconst": cconst,
            "b2": b2,
        })
    return in_maps


def _run(in_maps, trace=False):
    from concourse.bass_utils import run_bass_kernel_spmd
    if "nc" not in _CACHE:
        _CACHE["nc"] = _build_nc()
    nc = _CACHE["nc"]
    _CACHE["last_nc"] = nc
    res = run_bass_kernel_spmd(nc, in_maps, core_ids=list(range(NCORES)),
                               trace=trace)
    return res


def kernel(local_feats, binary_feats, sparse_idx, W1, b1, W2, b2):
    local_feats = np.ascontiguousarray(local_feats, dtype=np.float32)
    in_maps = _prep_inputs(local_feats, binary_feats, W1, b1, W2, b2)
    res = _run(in_maps)
    G = np.zeros((B, N, H), dtype=np.float32)
    for c in range(NCORES):
        G[c * BPC:(c + 1) * BPC] = np.asarray(
            res.results[c]["G"], dtype=np.float32).reshape(BPC, N, H)
    sparse_idx = np.asarray(sparse_idx)
    bb = sparse_idx[:, 0].astype(np.int64)
    ii = sparse_idx[:, 1].astype(np.int64)
    jj = sparse_idx[:, 2].astype(np.int64)
    lp = local_feats[bb, ii] + local_feats[bb, jj]
    gp = G[bb, ii] + G[bb, jj]
    return (lp, gp)
